# revision 1
# baseline (speedup 1.0000x reference)
"""Trainium2 Bass kernel for nn_AttentionHead (B=4, S=4096, D_IN=1024, DK=DV=64).

Sharding: 8 cores = batch(4) x query-half(2). Each core computes attention for
its 2048 query rows against the full 4096-key sequence of its batch. No
collectives.

Per-core algorithm (all matmul compute in bf16, f32 accumulation):
  1. Load raw q/k/v with a 32x32 block-swizzle cast-DMA (f32 DRAM -> bf16 SBUF),
     then DVE stream-transpose to get x^T tiles [128 d_in, seq] in SBUF.
  2. Projections with W stationary: qT = Wq^T q^T [64, 2048], kT [64, 4096],
     vT [64, 4096]; bias added on PSUM eviction (per-partition scalar).
  3. v1 = PE-transpose of vT -> v natural [kv, 64] with a ones column appended
     (column 64), so the PV matmul also accumulates the softmax denominator.
  4. scoresT[kv, q] = kT_tile^T qT (K=64), exp via ScalarE (scale=1/8) into
     bf16 expT tiles, PV: outT[65, 512] += v1_chunk^T expT (K=128 kv).
  5. Finalize: PE-transpose outT -> [128 q, 65], reciprocal of col 64,
     per-partition scale, DMA out f32.
"""
import os
import numpy as np

import concourse.bass as bass
import concourse.mybir as mybir
import concourse.tile as tile
from concourse import bacc
from concourse.bass_utils import run_bass_kernel_spmd
from concourse.masks import make_identity

F32 = mybir.dt.float32
BF16 = mybir.dt.bfloat16
AX = mybir.AxisListType.X
EXP = mybir.ActivationFunctionType.Exp

B, S, D_IN, DK, DV = 4, 4096, 1024, 64, 64
SQ = S // 2            # 2048 query rows per core
NCH = D_IN // 128      # 8 d_in chunks
NKV = S // 128         # 32 kv tiles
NQB = SQ // 512        # 4 query blocks of 512
PASS = 2048            # seq rows per load pass

_NC_CACHE = {}


def _load_pass(nc, Ap, Bp, x_ext, s0, ext_rows):
    """Swizzle cast-load + stream-transpose one pass of PASS seq rows.

    Returns list of 8 bf16 tiles Bs[c] = x[s0:s0+PASS, 128c:128c+128]^T with
    layout [128 d, PASS seq].
    """
    Bs = []
    for c in range(NCH):
        d0 = 128 * c
        A = Ap.tile([128, PASS], BF16, tag="A")
        for db in range(4):
            xin = x_ext[s0 : s0 + PASS, d0 + 32 * db : d0 + 32 * db + 32].rearrange(
                "(sb i) j -> i sb j", i=32
            )
            nc.gpsimd.dma_start(out=A[32 * db : 32 * db + 32, :], in_=xin)
        Bt = Bp.tile([128, PASS], BF16, tag="B")
        nc.vector.transpose(out=Bt[:, :], in_=A[:, :])
        Bs.append(Bt)
    return Bs


def _project(nc, pp, Bs, W, bias_t, outT, col0):
    """outT[:, col0:col0+PASS] (bf16 [64, *]) = W^T x^T + bias, accumulating
    over the 8 d_in chunks in PSUM per 512-block."""
    for sb in range(PASS // 512):
        ps = pp.tile([64, 512], F32, tag="pp")
        for c in range(NCH):
            nc.tensor.matmul(
                ps[:, :],
                W[:, c, :],
                Bs[c][:, 512 * sb : 512 * (sb + 1)],
                start=(c == 0),
                stop=(c == NCH - 1),
            )
        nc.vector.tensor_scalar_add(
            outT[:, col0 + 512 * sb : col0 + 512 * (sb + 1)], ps[:, :], bias_t[:, :]
        )


def build_attention_nc():
    nc = bacc.Bacc()

    q_ext = nc.declare_dram_parameter("q", [SQ, D_IN], F32, isOutput=False)
    k_ext = nc.declare_dram_parameter("k", [S, D_IN], F32, isOutput=False)
    v_ext = nc.declare_dram_parameter("v", [S, D_IN], F32, isOutput=False)
    wq_ext = nc.declare_dram_parameter("wq", [D_IN, DK], F32, isOutput=False)
    wk_ext = nc.declare_dram_parameter("wk", [D_IN, DK], F32, isOutput=False)
    wv_ext = nc.declare_dram_parameter("wv", [D_IN, DV], F32, isOutput=False)
    bq_ext = nc.declare_dram_parameter("bq", [DK], F32, isOutput=False)
    bk_ext = nc.declare_dram_parameter("bk", [DK], F32, isOutput=False)
    bv_ext = nc.declare_dram_parameter("bv", [DV], F32, isOutput=False)
    out_ext = nc.declare_dram_parameter("out", [SQ, DV], F32, isOutput=True)

    with tile.TileContext(nc) as tc:
        with (
            tc.tile_pool(name="single", bufs=1) as sg,
            tc.tile_pool(name="Ap", bufs=3) as Ap,
            tc.tile_pool(name="Bp", bufs=10) as Bp,
            tc.tile_pool(name="expp", bufs=4) as expp,
            tc.tile_pool(name="fin", bufs=2) as fin,
            tc.tile_pool(name="pp", bufs=2, space="PSUM") as pp,
            tc.tile_pool(name="sc", bufs=2, space="PSUM") as sc,
            tc.tile_pool(name="ot", bufs=4, space="PSUM") as ot,
        ):
            # ---- constants
            ident_b = sg.tile([128, 128], BF16)
            make_identity(nc, ident_b[:, :])
            ident_f = sg.tile([128, 128], F32)
            make_identity(nc, ident_f[:, :])

            # weights -> bf16 [128, 8, 64] (cast during DMA)
            Wq = sg.tile([128, NCH, DK], BF16)
            Wk = sg.tile([128, NCH, DK], BF16)
            Wv = sg.tile([128, NCH, DV], BF16)
            for W, ext in ((Wq, wq_ext), (Wk, wk_ext), (Wv, wv_ext)):
                nc.gpsimd.dma_start(
                    out=W[:, :, :], in_=ext.rearrange("(c p) n -> p c n", p=128)
                )
            bq_t = sg.tile([64, 1], F32)
            bk_t = sg.tile([64, 1], F32)
            bv_t = sg.tile([64, 1], F32)
            for bt, ext in ((bq_t, bq_ext), (bk_t, bk_ext), (bv_t, bv_ext)):
                nc.sync.dma_start(out=bt[:, :], in_=ext[:].unsqueeze(-1))

            # projected tensors (bf16)
            qT = sg.tile([64, SQ], BF16)    # [dk, q]
            kT = sg.tile([64, S], BF16)     # [dk, kv]
            vT = sg.tile([64, S], BF16)     # [dv, kv]
            v1 = sg.tile([128, NKV, DV + 1], BF16)  # v natural + ones col
            nc.vector.memset(v1[:, :, DV : DV + 1], 1.0)

            # prime PE's observed clock with the gpsimd tick (identity)
            prime_ps = pp.tile([128, 128], BF16, tag="pp")
            nc.tensor.transpose(prime_ps[:, :], ident_b[:, :], ident_b[:, :])

            # outT accumulators, one per query block [65, 512] f32
            otps = [ot.tile([DV + 1, 512], F32, tag="ot", name=f"otps{i}") for i in range(NQB)]

            def v_flip(c):
                # vT[:, 128c:128c+128] -> v1[:, c, :64]
                ps = pp.tile([128, DV], BF16, tag="pp")
                nc.tensor.transpose(
                    ps[:, :], vT[:, 128 * c : 128 * (c + 1)], ident_b[0:64, 0:64]
                )
                nc.scalar.copy(v1[:, c, 0:DV], ps[:, :])

            def phase_b(c):
                # scoresT + exp + PV for kv chunk c against all query blocks
                for qb in range(NQB):
                    sps = sc.tile([128, 512], F32, tag="sc")
                    nc.tensor.matmul(
                        sps[:, :],
                        kT[:, 128 * c : 128 * (c + 1)],
                        qT[:, 512 * qb : 512 * (qb + 1)],
                        start=True,
                        stop=True,
                    )
                    ex = expp.tile([128, 512], BF16, tag="ex")
                    nc.scalar.activation(
                        out=ex[:, :], in_=sps[:, :], func=EXP, scale=0.125
                    )
                    nc.tensor.matmul(
                        otps[qb][:, :],
                        v1[:, c, :],
                        ex[:, :],
                        start=(c == 0),
                        stop=(c == NKV - 1),
                    )

            # ---- phase A part 1: q, k half 1, v half 1
            Bs = _load_pass(nc, Ap, Bp, q_ext, 0, SQ)
            _project(nc, pp, Bs, Wq, bq_t, qT, 0)
            Bs = _load_pass(nc, Ap, Bp, k_ext, 0, S)
            _project(nc, pp, Bs, Wk, bk_t, kT, 0)
            Bs = _load_pass(nc, Ap, Bp, v_ext, 0, S)
            _project(nc, pp, Bs, Wv, bv_t, vT, 0)
            for c in range(NKV // 2):
                v_flip(c)

            # ---- phase B half 1 (kv chunks 0..15)
            for c in range(NKV // 2):
                phase_b(c)

            # ---- phase A part 2: k half 2, v half 2
            Bs = _load_pass(nc, Ap, Bp, k_ext, PASS, S)
            _project(nc, pp, Bs, Wk, bk_t, kT, PASS)
            Bs = _load_pass(nc, Ap, Bp, v_ext, PASS, S)
            _project(nc, pp, Bs, Wv, bv_t, vT, PASS)
            for c in range(NKV // 2, NKV):
                v_flip(c)

            # ---- phase B half 2
            for c in range(NKV // 2, NKV):
                phase_b(c)

            # ---- finalize: normalize + transpose back + store
            for qb in range(NQB):
                o_sb = fin.tile([DV + 1, 512], F32, tag="osb")
                nc.vector.tensor_copy(o_sb[:, :], otps[qb][:, :])
                for t in range(4):
                    tp = pp.tile([128, DV + 1], F32, tag="pp")
                    nc.tensor.transpose(
                        tp[:, :],
                        o_sb[:, 128 * t : 128 * (t + 1)],
                        ident_f[0 : DV + 1, 0 : DV + 1],
                    )
                    rec = fin.tile([128, 1], F32, tag="rec")
                    nc.vector.reciprocal(rec[:, :], tp[:, DV : DV + 1])
                    o_f = fin.tile([128, DV], F32, tag="of")
                    nc.vector.tensor_scalar_mul(o_f[:, :], tp[:, 0:DV], rec[:, :])
                    nc.scalar.dma_start(
                        out=out_ext[512 * qb + 128 * t : 512 * qb + 128 * (t + 1), :],
                        in_=o_f[:, :],
                    )

    nc.compile()
    return nc


def _get_nc():
    if "nc" not in _NC_CACHE:
        _NC_CACHE["nc"] = build_attention_nc()
    return _NC_CACHE["nc"]


def kernel(query, key, value, Wq, bq, Wk, bk, Wv, bv):
    query = np.asarray(query, dtype=np.float32)
    key = np.asarray(key, dtype=np.float32)
    value = np.asarray(value, dtype=np.float32)
    wq = np.ascontiguousarray(np.asarray(Wq, np.float32))
    wk = np.ascontiguousarray(np.asarray(Wk, np.float32))
    wv = np.ascontiguousarray(np.asarray(Wv, np.float32))
    bq_ = np.ascontiguousarray(np.asarray(bq, np.float32))
    bk_ = np.ascontiguousarray(np.asarray(bk, np.float32))
    bv_ = np.ascontiguousarray(np.asarray(bv, np.float32))

    in_maps = []
    for b in range(B):
        for h in range(2):
            in_maps.append(
                {
                    "q": np.ascontiguousarray(query[b, h * SQ : (h + 1) * SQ]),
                    "k": np.ascontiguousarray(key[b]),
                    "v": np.ascontiguousarray(value[b]),
                    "wq": wq, "wk": wk, "wv": wv,
                    "bq": bq_, "bk": bk_, "bv": bv_,
                }
            )

    nc = _get_nc()
    trace = bool(int(os.environ.get("BASS_KERNEL_TRACE", "0")))
    res = run_bass_kernel_spmd(nc, in_maps, core_ids=list(range(8)), trace=trace)
    _NC_CACHE["last_results"] = res

    out = np.empty((B, S, DV), np.float32)
    for b in range(B):
        for h in range(2):
            out[b, h * SQ : (h + 1) * SQ] = res.results[2 * b + h]["out"]
    return out



# revision 6
# speedup vs baseline: 1.3734x; 1.3734x over previous
"""Trainium2 Bass kernel for nn_AttentionHead (B=4, S=4096, D_IN=1024, DK=DV=64).

Sharding: 8 cores = batch(4) x query-half(2). Each core computes attention for
its 2048 query rows against the full 4096-key sequence of its batch. No
collectives.

Per-core algorithm (matmul compute in bf16, f32 accumulation):
  1. Natural-layout cast-DMA loads (f32 DRAM -> bf16 SBUF, 4KB-contiguous
     rows, descriptor-efficient): x tiles [128 seq, 1024 d].
  2. x^T via PE transposes ([128,128] blocks -> PSUM, engine copy-back to
     SBUF), software-pipelined with the projection matmuls.
  3. Projections W-stationary: qT [64, 2048], kT [64, 4096], vT per-block
     [64, 512]; bias added on PSUM eviction. vT is PE-flipped to natural
     v1 [kv, 65] with a ones column (col 64) so PV also accumulates the
     softmax denominator.
  4. Streaming attention: per kv chunk, scoresT = kT_c^T qT for all 4 query
     blocks; exp via ScalarE (scale=1/8). PV accumulates in PSUM for query
     blocks 0-1 immediately; exp tiles for blocks 2-3 are kept in SBUF and
     their PV runs as a dense sweep afterwards (PSUM has only 2 free banks
     for output accumulators).
  5. Finalize: PE-transpose out^T -> [128 q, 65], reciprocal of col 64,
     per-partition scale, DMA out f32.
"""
import os
import numpy as np

import concourse.bass as bass
import concourse.mybir as mybir
import concourse.tile as tile
from concourse import bacc
from concourse.bass_utils import run_bass_kernel_spmd
from concourse.masks import make_identity

F32 = mybir.dt.float32
BF16 = mybir.dt.bfloat16
EXP = mybir.ActivationFunctionType.Exp

B, S, D_IN, DK, DV = 4, 4096, 1024, 64, 64
SQ = S // 2            # 2048 query rows per core
NCH = D_IN // 128      # 8 d_in chunks
NKV = S // 128         # 32 kv tiles
NQB = SQ // 512        # 4 query blocks of 512
NKB = S // 512         # 8 kv blocks of 512

_NC_CACHE = {}


def build_attention_nc():
    nc = bacc.Bacc()

    q_ext = nc.declare_dram_parameter("q", [SQ, D_IN], F32, isOutput=False)
    k_ext = nc.declare_dram_parameter("k", [S, D_IN], F32, isOutput=False)
    v_ext = nc.declare_dram_parameter("v", [S, D_IN], F32, isOutput=False)
    wq_ext = nc.declare_dram_parameter("wq", [D_IN, DK], F32, isOutput=False)
    wk_ext = nc.declare_dram_parameter("wk", [D_IN, DK], F32, isOutput=False)
    wv_ext = nc.declare_dram_parameter("wv", [D_IN, DV], F32, isOutput=False)
    bq_ext = nc.declare_dram_parameter("bq", [DK], F32, isOutput=False)
    bk_ext = nc.declare_dram_parameter("bk", [DK], F32, isOutput=False)
    bv_ext = nc.declare_dram_parameter("bv", [DV], F32, isOutput=False)
    out_ext = nc.declare_dram_parameter("out", [SQ, DV], F32, isOutput=True)

    with tile.TileContext(nc) as tc:
        with (
            tc.tile_pool(name="single", bufs=1) as sg,
            tc.tile_pool(name="xn", bufs=4) as xn,
            tc.tile_pool(name="xtp", bufs=6) as xtp,
            tc.tile_pool(name="exg", bufs=4) as exg,
            tc.tile_pool(name="vtp", bufs=2) as vtp,
            tc.tile_pool(name="fin", bufs=2) as fin,
            tc.tile_pool(name="otp", bufs=2, space="PSUM") as otp,
            tc.tile_pool(name="scp", bufs=3, space="PSUM") as scp,
            tc.tile_pool(name="trp", bufs=2, space="PSUM") as trp,
            tc.tile_pool(name="pjp", bufs=1, space="PSUM") as pjp,
        ):
            # ---- constants
            ident_b = sg.tile([128, 128], BF16)
            make_identity(nc, ident_b[:, :])
            ident_f = sg.tile([128, 128], F32)
            make_identity(nc, ident_f[:, :])

            # weights -> bf16 [128, 8, 64] (cast during DMA)
            Wq = sg.tile([128, NCH, DK], BF16)
            Wk = sg.tile([128, NCH, DK], BF16)
            Wv = sg.tile([128, NCH, DV], BF16)
            for W, ext in ((Wq, wq_ext), (Wk, wk_ext), (Wv, wv_ext)):
                nc.gpsimd.dma_start(
                    out=W[:, :, :], in_=ext.rearrange("(c p) n -> p c n", p=128)
                )
            bq_t = sg.tile([64, 1], F32)
            bk_t = sg.tile([64, 1], F32)
            bv_t = sg.tile([64, 1], F32)
            for bt, ext in ((bq_t, bq_ext), (bk_t, bk_ext), (bv_t, bv_ext)):
                nc.sync.dma_start(out=bt[:, :], in_=ext[:].unsqueeze(-1))

            # projected tensors (bf16)
            qT = sg.tile([64, SQ], BF16)    # [dk, q]
            kT = sg.tile([64, S], BF16)     # [dk, kv]
            v1 = sg.tile([128, NKV, DV + 1], BF16)  # v natural + ones col
            nc.vector.memset(v1[:, :, DV : DV + 1], 1.0)
            # exp tiles for query blocks 2-3, PV-ed after the kv stream
            ex2 = sg.tile([128, 2 * NKV, 512], BF16)

            # prime the PE clock
            prime_ps = trp.tile([128, 128], BF16, tag="tr")
            nc.tensor.transpose(prime_ps[:, :], ident_b[:, :], ident_b[:, :])

            # round-robin copy engines for PSUM->SBUF evictions. GPSIMD cannot
            # read PSUM, so split between DVE and Act (Copy shares the Exp
            # activation table set -> no table reloads).
            cp_state = {"i": 0}

            def eng_copy(dst, src):
                i = cp_state["i"]
                cp_state["i"] += 1
                if i % 3 == 2:
                    nc.scalar.copy(dst, src)
                else:
                    nc.vector.tensor_copy(dst, src)

            def load_block(x_ext, s0):
                """One 512-row natural-layout cast load -> [128, 4, 1024]."""
                xt = xn.tile([128, 4, D_IN], BF16, tag="xn", name="xnt")
                nc.gpsimd.dma_start(
                    out=xt[:, :, :],
                    in_=x_ext[s0 : s0 + 512, :].rearrange("(t p) d -> p t d", p=128),
                )
                return xt

            DELAY = 3  # chunks between transpose-group and its projection

            def block_project(xnt, W, bias_t, outT, col0):
                """outT[:, col0:col0+512] = (x_block W + b)^T via PE transposes
                + chunk-accumulated projection, software-pipelined."""
                pj = pjp.tile([64, 512], F32, tag="pj")
                xts = [None] * NCH

                def emit_trans(c):
                    tr = trp.tile([128, 512], BF16, tag="tr", name="tr")
                    xt = xtp.tile([128, 512], BF16, tag="xt", name="xt")
                    for t in range(4):
                        nc.tensor.transpose(
                            tr[:, 128 * t : 128 * (t + 1)],
                            xnt[:, t, 128 * c : 128 * (c + 1)],
                            ident_b[:, :],
                        )
                        if t == 1:
                            eng_copy(xt[:, 0:256], tr[:, 0:256])
                    eng_copy(xt[:, 256:512], tr[:, 256:512])
                    xts[c] = xt

                def emit_proj(m):
                    nc.tensor.matmul(
                        pj[:, :],
                        W[:, m, :],
                        xts[m][:, :],
                        start=(m == 0),
                        stop=(m == NCH - 1),
                    )

                for c in range(NCH):
                    emit_trans(c)
                    if c >= DELAY:
                        emit_proj(c - DELAY)
                for m in range(NCH - DELAY, NCH):
                    emit_proj(m)
                nc.vector.tensor_scalar_add(
                    outT[:, col0 : col0 + 512], pj[:, :], bias_t[:, :]
                )

            def flips(vt_blk, b):
                """vT block [64, 512] -> natural v1[:, 4b:4b+4, :64]."""
                tr = trp.tile([128, 256], BF16, tag="tr", name="trf")
                for j in range(4):
                    nc.tensor.transpose(
                        tr[:, 64 * j : 64 * (j + 1)],
                        vt_blk[:, 128 * j : 128 * (j + 1)],
                        ident_b[0:64, 0:64],
                    )
                nc.vector.tensor_copy(
                    v1[:, 4 * b : 4 * b + 4, 0:DV],
                    tr[:, 0:256].rearrange("p (j v) -> p j v", j=4),
                )

            def attn_chunk(c, ots):
                """scoresT + exp for all 4 q blocks; PV for blocks 0-1."""
                exs = []
                for qb in range(NQB):
                    sp = scp.tile([128, 512], F32, tag="sc", name="sp")
                    nc.tensor.matmul(
                        sp[:, :],
                        kT[:, 128 * c : 128 * (c + 1)],
                        qT[:, 512 * qb : 512 * (qb + 1)],
                        start=True,
                        stop=True,
                    )
                    if qb < 2:
                        ex = exg.tile([128, 512], BF16, tag="ex", name="ex")[:, :]
                    else:
                        ex = ex2[:, 2 * c + (qb - 2), :]
                    nc.scalar.activation(out=ex, in_=sp[:, :], func=EXP, scale=0.125)
                    exs.append(ex)
                for qb in range(2):
                    nc.tensor.matmul(
                        ots[qb][:, :],
                        v1[:, c, :],
                        exs[qb],
                        start=(c == 0),
                        stop=(c == NKV - 1),
                    )

            def finalize(ot, qb):
                o_sb = fin.tile([DV + 1, 512], F32, tag="osb", name="osb")
                nc.vector.tensor_copy(o_sb[:, :], ot[:, :])
                for t in range(4):
                    tp = trp.tile([128, DV + 1], F32, tag="tr", name="tp")
                    nc.tensor.transpose(
                        tp[:, :],
                        o_sb[:, 128 * t : 128 * (t + 1)],
                        ident_f[0 : DV + 1, 0 : DV + 1],
                    )
                    rec = fin.tile([128, 1], F32, tag="rec", name="rec")
                    nc.vector.reciprocal(rec[:, :], tp[:, DV : DV + 1])
                    o_f = fin.tile([128, DV], F32, tag="of", name="of")
                    nc.vector.tensor_scalar_mul(o_f[:, :], tp[:, 0:DV], rec[:, :])
                    nc.sync.dma_start(
                        out=out_ext[512 * qb + 128 * t : 512 * qb + 128 * (t + 1), :],
                        in_=o_f[:, :],
                    )

            # ---- Q phase: project all 2048 query rows
            for qb in range(NQB):
                xnt = load_block(q_ext, 512 * qb)
                block_project(xnt, Wq, bq_t, qT, 512 * qb)

            # ---- KV stream: load/project k,v per 512-block, attention per chunk
            ots = [
                otp.tile([DV + 1, 512], F32, tag="ot", name=f"ot{i}") for i in range(2)
            ]
            for b in range(NKB):
                xk = load_block(k_ext, 512 * b)
                xv = load_block(v_ext, 512 * b)
                block_project(xk, Wk, bk_t, kT, 512 * b)
                vt = vtp.tile([64, 512], BF16, tag="vt", name="vt")
                block_project(xv, Wv, bv_t, vt, 0)
                flips(vt, b)
                for j in range(4):
                    attn_chunk(4 * b + j, ots)

            # ---- finalize q blocks 0-1, then dense PV sweep for blocks 2-3
            finalize(ots[0], 0)
            finalize(ots[1], 1)
            ots2 = [
                otp.tile([DV + 1, 512], F32, tag="ot", name=f"ot2{i}")
                for i in range(2)
            ]
            for g in range(2):
                for c in range(NKV):
                    nc.tensor.matmul(
                        ots2[g][:, :],
                        v1[:, c, :],
                        ex2[:, 2 * c + g, :],
                        start=(c == 0),
                        stop=(c == NKV - 1),
                    )
            finalize(ots2[0], 2)
            finalize(ots2[1], 3)

    nc.compile()
    return nc


def _get_nc():
    if "nc" not in _NC_CACHE:
        _NC_CACHE["nc"] = build_attention_nc()
    return _NC_CACHE["nc"]


def kernel(query, key, value, Wq, bq, Wk, bk, Wv, bv):
    query = np.asarray(query, dtype=np.float32)
    key = np.asarray(key, dtype=np.float32)
    value = np.asarray(value, dtype=np.float32)
    wq = np.ascontiguousarray(np.asarray(Wq, np.float32))
    wk = np.ascontiguousarray(np.asarray(Wk, np.float32))
    wv = np.ascontiguousarray(np.asarray(Wv, np.float32))
    bq_ = np.ascontiguousarray(np.asarray(bq, np.float32))
    bk_ = np.ascontiguousarray(np.asarray(bk, np.float32))
    bv_ = np.ascontiguousarray(np.asarray(bv, np.float32))

    in_maps = []
    for b in range(B):
        for h in range(2):
            in_maps.append(
                {
                    "q": np.ascontiguousarray(query[b, h * SQ : (h + 1) * SQ]),
                    "k": np.ascontiguousarray(key[b]),
                    "v": np.ascontiguousarray(value[b]),
                    "wq": wq, "wk": wk, "wv": wv,
                    "bq": bq_, "bk": bk_, "bv": bv_,
                }
            )

    nc = _get_nc()
    trace = bool(int(os.environ.get("BASS_KERNEL_TRACE", "0")))
    res = run_bass_kernel_spmd(nc, in_maps, core_ids=list(range(8)), trace=trace)
    _NC_CACHE["last_results"] = res

    out = np.empty((B, S, DV), np.float32)
    for b in range(B):
        for h in range(2):
            out[b, h * SQ : (h + 1) * SQ] = res.results[2 * b + h]["out"]
    return out


# revision 9
# speedup vs baseline: 1.4844x; 1.0808x over previous
"""Trainium2 Bass kernel for nn_AttentionHead (B=4, S=4096, D_IN=1024, DK=DV=64).

Sharding: 8 cores = batch(4) x query-half(2). Each core computes attention for
its 2048 query rows against the full 4096-key sequence of its batch. No
collectives.

Per-core algorithm (matmul compute in bf16, f32 accumulation):
  1. Natural-layout cast-DMA loads (f32 DRAM -> bf16 SBUF, 4KB-contiguous
     rows, descriptor-efficient): x tiles [128 seq, 1024 d].
  2. x^T via PE transposes ([128,128] blocks -> PSUM, engine copy-back to
     SBUF), software-pipelined with the projection matmuls.
  3. Projections W-stationary: qT [64, 2048], kT [64, 4096], vT per-block
     [64, 512]; bias added on PSUM eviction. vT is PE-flipped to natural
     v1 [kv, 65] with a ones column (col 64) so PV also accumulates the
     softmax denominator.
  4. Streaming attention: per kv chunk, scoresT = kT_c^T qT for all 4 query
     blocks; exp via ScalarE (scale=1/8). PV accumulates in PSUM for query
     blocks 0-1 immediately; exp tiles for blocks 2-3 are kept in SBUF and
     their PV runs as a dense sweep afterwards (PSUM has only 2 free banks
     for output accumulators).
  5. Finalize: PE-transpose out^T -> [128 q, 65], reciprocal of col 64,
     per-partition scale, DMA out f32.
"""
import os
import numpy as np

import concourse.bass as bass
import concourse.mybir as mybir
import concourse.tile as tile
from concourse import bacc
from concourse.bass_utils import run_bass_kernel_spmd
from concourse.masks import make_identity

F32 = mybir.dt.float32
BF16 = mybir.dt.bfloat16
EXP = mybir.ActivationFunctionType.Exp

B, S, D_IN, DK, DV = 4, 4096, 1024, 64, 64
SQ = S // 2            # 2048 query rows per core
NCH = D_IN // 128      # 8 d_in chunks
NKV = S // 128         # 32 kv tiles
NQB = SQ // 512        # 4 query blocks of 512
NKB = S // 512         # 8 kv blocks of 512

_NC_CACHE = {}


def build_attention_nc():
    nc = bacc.Bacc()

    q_ext = nc.declare_dram_parameter("q", [SQ, D_IN], F32, isOutput=False)
    k_ext = nc.declare_dram_parameter("k", [S, D_IN], F32, isOutput=False)
    v_ext = nc.declare_dram_parameter("v", [S, D_IN], F32, isOutput=False)
    wq_ext = nc.declare_dram_parameter("wq", [D_IN, DK], F32, isOutput=False)
    wk_ext = nc.declare_dram_parameter("wk", [D_IN, DK], F32, isOutput=False)
    wv_ext = nc.declare_dram_parameter("wv", [D_IN, DV], F32, isOutput=False)
    bq_ext = nc.declare_dram_parameter("bq", [DK], F32, isOutput=False)
    bk_ext = nc.declare_dram_parameter("bk", [DK], F32, isOutput=False)
    bv_ext = nc.declare_dram_parameter("bv", [DV], F32, isOutput=False)
    out_ext = nc.declare_dram_parameter("out", [SQ, DV], F32, isOutput=True)

    with tile.TileContext(nc) as tc:
        with (
            tc.tile_pool(name="single", bufs=1) as sg,
            tc.tile_pool(name="xn", bufs=6) as xn,
            tc.tile_pool(name="xtp", bufs=6) as xtp,
            tc.tile_pool(name="exg", bufs=4) as exg,
            tc.tile_pool(name="vtp", bufs=2) as vtp,
            tc.tile_pool(name="fin", bufs=2) as fin,
            tc.tile_pool(name="otp", bufs=2, space="PSUM") as otp,
            tc.tile_pool(name="scp", bufs=3, space="PSUM") as scp,
            tc.tile_pool(name="trp", bufs=2, space="PSUM") as trp,
            tc.tile_pool(name="pjp", bufs=1, space="PSUM") as pjp,
        ):
            # ---- constants
            ident_b = sg.tile([128, 128], BF16)
            make_identity(nc, ident_b[:, :])
            ident_f = sg.tile([128, 128], F32)
            make_identity(nc, ident_f[:, :])

            # weights -> bf16 [128, 8, 64] (cast during DMA)
            Wq = sg.tile([128, NCH, DK], BF16)
            Wk = sg.tile([128, NCH, DK], BF16)
            Wv = sg.tile([128, NCH, DV], BF16)
            for W, ext in ((Wq, wq_ext), (Wk, wk_ext), (Wv, wv_ext)):
                nc.gpsimd.dma_start(
                    out=W[:, :, :], in_=ext.rearrange("(c p) n -> p c n", p=128)
                )
            bq_t = sg.tile([64, 1], F32)
            bk_t = sg.tile([64, 1], F32)
            bv_t = sg.tile([64, 1], F32)
            for bt, ext in ((bq_t, bq_ext), (bk_t, bk_ext), (bv_t, bv_ext)):
                nc.sync.dma_start(out=bt[:, :], in_=ext[:].unsqueeze(-1))

            # projected tensors (bf16)
            qT = sg.tile([64, SQ], BF16)    # [dk, q]
            kT = sg.tile([64, S], BF16)     # [dk, kv]
            v1 = sg.tile([128, NKV, DV + 1], BF16)  # v natural + ones col
            nc.vector.memset(v1[:, :, DV : DV + 1], 1.0)
            # exp tiles for query blocks 2-3, PV-ed after the kv stream
            ex2 = sg.tile([128, 2 * NKV, 512], BF16)

            # prime the PE clock
            prime_ps = trp.tile([128, 128], BF16, tag="tr")
            nc.tensor.transpose(prime_ps[:, :], ident_b[:, :], ident_b[:, :])

            # round-robin copy engines for PSUM->SBUF evictions. GPSIMD cannot
            # read PSUM, so split between DVE and Act (Copy shares the Exp
            # activation table set -> no table reloads).
            cp_state = {"i": 0}

            def eng_copy(dst, src):
                i = cp_state["i"]
                cp_state["i"] += 1
                if i % 3 == 2:
                    nc.scalar.copy(dst, src)
                else:
                    nc.vector.tensor_copy(dst, src)

            def load_block(x_ext, s0):
                """One 512-row natural-layout cast load -> [128, 4, 1024].
                Split in two DMAs so the first tiles land sooner."""
                xt = xn.tile([128, 4, D_IN], BF16, tag="xn", name="xnt")
                for h in range(2):
                    nc.gpsimd.dma_start(
                        out=xt[:, 2 * h : 2 * h + 2, :],
                        in_=x_ext[s0 + 256 * h : s0 + 256 * (h + 1), :].rearrange(
                            "(t p) d -> p t d", p=128
                        ),
                    )
                return xt

            def interleave(prod_units, cons_units):
                """Emit producer thunks, sprinkling consumer thunks evenly."""
                np_, nc_ = len(prod_units), len(cons_units)
                ci = 0
                for pi, u in enumerate(prod_units):
                    u()
                    while ci * np_ < (pi + 1) * nc_:
                        cons_units[ci]()
                        ci += 1
                while ci < nc_:
                    cons_units[ci]()
                    ci += 1

            DELAY = 3  # chunks between transpose-group and its projection

            def prod_block(xnt, W, bias_t, outT, col0):
                """Thunks producing outT[:, col0:col0+512] = (x_block W + b)^T
                via PE transposes + chunk-accumulated projection."""
                st = {"pj": None}
                xts = [None] * NCH

                def trans_unit(c):
                    def f():
                        tr = trp.tile([128, 512], BF16, tag="tr", name="tr")
                        xt = xtp.tile([128, 512], BF16, tag="xt", name="xt")
                        for t in range(4):
                            nc.tensor.transpose(
                                tr[:, 128 * t : 128 * (t + 1)],
                                xnt[:, t, 128 * c : 128 * (c + 1)],
                                ident_b[:, :],
                            )
                            if t == 1:
                                eng_copy(xt[:, 0:256], tr[:, 0:256])
                        eng_copy(xt[:, 256:512], tr[:, 256:512])
                        xts[c] = xt

                    return f

                def proj_unit(m):
                    def f():
                        if st["pj"] is None:
                            st["pj"] = pjp.tile([64, 512], F32, tag="pj", name="pj")
                        nc.tensor.matmul(
                            st["pj"][:, :],
                            W[:, m, :],
                            xts[m][:, :],
                            start=(m == 0),
                            stop=(m == NCH - 1),
                        )

                    return f

                def bias_unit():
                    nc.vector.tensor_scalar_add(
                        outT[:, col0 : col0 + 512], st["pj"][:, :], bias_t[:, :]
                    )

                units = []
                for c in range(NCH):
                    units.append(trans_unit(c))
                    if c >= DELAY:
                        units.append(proj_unit(c - DELAY))
                for m in range(NCH - DELAY, NCH):
                    units.append(proj_unit(m))
                units.append(bias_unit)
                return units

            def flips_unit(vt_blk, b):
                """vT block [64, 512] -> natural v1[:, 4b:4b+4, :64]."""

                def f():
                    tr = trp.tile([128, 256], BF16, tag="tr", name="trf")
                    for j in range(4):
                        nc.tensor.transpose(
                            tr[:, 64 * j : 64 * (j + 1)],
                            vt_blk[:, 128 * j : 128 * (j + 1)],
                            ident_b[0:64, 0:64],
                        )
                    nc.vector.tensor_copy(
                        v1[:, 4 * b : 4 * b + 4, 0:DV],
                        tr[:, 0:256].rearrange("p (j v) -> p j v", j=4),
                    )

                return f

            def cons_block(b, ots):
                """Attention thunks for kv block b: scoresT+exp for all 4 q
                blocks, immediate PV for q blocks 0-1."""
                exd = {}
                units = []
                for j in range(4):
                    c = 4 * b + j

                    def sc_unit(c, qb):
                        def f():
                            sp = scp.tile([128, 512], F32, tag="sc", name="sp")
                            nc.tensor.matmul(
                                sp[:, :],
                                kT[:, 128 * c : 128 * (c + 1)],
                                qT[:, 512 * qb : 512 * (qb + 1)],
                                start=True,
                                stop=True,
                            )
                            if qb < 2:
                                ex = exg.tile(
                                    [128, 512], BF16, tag="ex", name="ex"
                                )[:, :]
                            else:
                                ex = ex2[:, 2 * c + (qb - 2), :]
                            nc.scalar.activation(
                                out=ex, in_=sp[:, :], func=EXP, scale=0.125
                            )
                            exd[(c, qb)] = ex

                        return f

                    def pv_unit(c, qb):
                        def f():
                            nc.tensor.matmul(
                                ots[qb][:, :],
                                v1[:, c, :],
                                exd[(c, qb)],
                                start=(c == 0),
                                stop=(c == NKV - 1),
                            )

                        return f

                    for qb in range(NQB):
                        units.append(sc_unit(c, qb))
                    units.append(pv_unit(c, 0))
                    units.append(pv_unit(c, 1))
                return units

            def finalize(ot, qb):
                o_sb = fin.tile([DV + 1, 512], F32, tag="osb", name="osb")
                nc.vector.tensor_copy(o_sb[:, :], ot[:, :])
                for t in range(4):
                    tp = scp.tile([128, DV + 1], F32, tag="sc", name="tp")
                    nc.tensor.transpose(
                        tp[:, :],
                        o_sb[:, 128 * t : 128 * (t + 1)],
                        ident_f[0 : DV + 1, 0 : DV + 1],
                    )
                    rec = fin.tile([128, 1], F32, tag="rec", name="rec")
                    nc.vector.reciprocal(rec[:, :], tp[:, DV : DV + 1])
                    o_f = fin.tile([128, DV], F32, tag="of", name="of")
                    nc.vector.tensor_scalar_mul(o_f[:, :], tp[:, 0:DV], rec[:, :])
                    nc.sync.dma_start(
                        out=out_ext[512 * qb + 128 * t : 512 * qb + 128 * (t + 1), :],
                        in_=o_f[:, :],
                    )

            # ---- Q phase: project all 2048 query rows
            for qb in range(NQB):
                xnt = load_block(q_ext, 512 * qb)
                interleave(prod_block(xnt, Wq, bq_t, qT, 512 * qb), [])

            # ---- KV stream: produce k/v block b while consuming attention of
            # block b-1 (keeps the PE stream dense so it holds peak p-state)
            ots = [
                otp.tile([DV + 1, 512], F32, tag="ot", name=f"ot{i}") for i in range(2)
            ]
            cons = []
            for b in range(NKB):
                xk = load_block(k_ext, 512 * b)
                xv = load_block(v_ext, 512 * b)
                vt = vtp.tile([64, 512], BF16, tag="vt", name="vt")
                prod = (
                    prod_block(xk, Wk, bk_t, kT, 512 * b)
                    + prod_block(xv, Wv, bv_t, vt, 0)
                    + [flips_unit(vt, b)]
                )
                interleave(prod, cons)
                cons = cons_block(b, ots)
            for u in cons:  # attention for the last kv block
                u()

            # ---- dense PV sweep for q blocks 2-3 (transpose banks are free
            # now), then all finalizes
            ots2 = [
                trp.tile([DV + 1, 512], F32, tag="tr", name=f"ot2{i}")
                for i in range(2)
            ]
            for g in range(2):
                for c in range(NKV):
                    nc.tensor.matmul(
                        ots2[g][:, :],
                        v1[:, c, :],
                        ex2[:, 2 * c + g, :],
                        start=(c == 0),
                        stop=(c == NKV - 1),
                    )
            finalize(ots[0], 0)
            finalize(ots[1], 1)
            finalize(ots2[0], 2)
            finalize(ots2[1], 3)

    nc.compile()
    return nc


def _get_nc():
    if "nc" not in _NC_CACHE:
        _NC_CACHE["nc"] = build_attention_nc()
    return _NC_CACHE["nc"]


def kernel(query, key, value, Wq, bq, Wk, bk, Wv, bv):
    query = np.asarray(query, dtype=np.float32)
    key = np.asarray(key, dtype=np.float32)
    value = np.asarray(value, dtype=np.float32)
    wq = np.ascontiguousarray(np.asarray(Wq, np.float32))
    wk = np.ascontiguousarray(np.asarray(Wk, np.float32))
    wv = np.ascontiguousarray(np.asarray(Wv, np.float32))
    bq_ = np.ascontiguousarray(np.asarray(bq, np.float32))
    bk_ = np.ascontiguousarray(np.asarray(bk, np.float32))
    bv_ = np.ascontiguousarray(np.asarray(bv, np.float32))

    in_maps = []
    for b in range(B):
        for h in range(2):
            in_maps.append(
                {
                    "q": np.ascontiguousarray(query[b, h * SQ : (h + 1) * SQ]),
                    "k": np.ascontiguousarray(key[b]),
                    "v": np.ascontiguousarray(value[b]),
                    "wq": wq, "wk": wk, "wv": wv,
                    "bq": bq_, "bk": bk_, "bv": bv_,
                }
            )

    nc = _get_nc()
    trace = bool(int(os.environ.get("BASS_KERNEL_TRACE", "0")))
    res = run_bass_kernel_spmd(nc, in_maps, core_ids=list(range(8)), trace=trace)
    _NC_CACHE["last_results"] = res

    out = np.empty((B, S, DV), np.float32)
    for b in range(B):
        for h in range(2):
            out[b, h * SQ : (h + 1) * SQ] = res.results[2 * b + h]["out"]
    return out


# revision 13
# speedup vs baseline: 1.6762x; 1.1292x over previous
"""Trainium2 Bass kernel for nn_AttentionHead (B=4, S=4096, D_IN=1024, DK=DV=64).

Sharding: 8 cores = batch(4) x query-half(2). Each core computes attention for
its 2048 query rows against the full 4096-key sequence of its batch. No
collectives.

Per-core algorithm (matmul compute in bf16, f32 accumulation):
  1. Natural-layout cast-DMA loads (f32 DRAM -> bf16 SBUF, 4KB-contiguous
     rows, descriptor-efficient): x tiles [128 seq, 1024 d].
  2. x^T via PE transposes ([128,128] blocks -> PSUM, engine copy-back to
     SBUF), software-pipelined with the projection matmuls.
  3. Projections W-stationary: qT [64, 2048], kT [64, 4096], vT per-block
     [64, 512]; bias added on PSUM eviction. vT is PE-flipped to natural
     v1 [kv, 65] with a ones column (col 64) so PV also accumulates the
     softmax denominator.
  4. Streaming attention: per kv chunk, scoresT = kT_c^T qT for all 4 query
     blocks; exp via ScalarE (scale=1/8). PV accumulates in PSUM for query
     blocks 0-1 immediately; exp tiles for blocks 2-3 are kept in SBUF and
     their PV runs as a dense sweep afterwards (PSUM has only 2 free banks
     for output accumulators).
  5. Finalize: PE-transpose out^T -> [128 q, 65], reciprocal of col 64,
     per-partition scale, DMA out f32.
"""
import os
import numpy as np

import concourse.bass as bass
import concourse.mybir as mybir
import concourse.tile as tile
from concourse import bacc
from concourse.bass_utils import run_bass_kernel_spmd
from concourse.masks import make_identity

F32 = mybir.dt.float32
BF16 = mybir.dt.bfloat16
EXP = mybir.ActivationFunctionType.Exp

B, S, D_IN, DK, DV = 4, 4096, 1024, 64, 64
SQ = S // 2            # 2048 query rows per core
NCH = D_IN // 128      # 8 d_in chunks
NKV = S // 128         # 32 kv tiles
NQB = SQ // 512        # 4 query blocks of 512
NKB = S // 512         # 8 kv blocks of 512

_NC_CACHE = {}


def build_attention_nc():
    nc = bacc.Bacc()

    q_ext = nc.declare_dram_parameter("q", [SQ, D_IN], F32, isOutput=False)
    k_ext = nc.declare_dram_parameter("k", [S, D_IN], F32, isOutput=False)
    v_ext = nc.declare_dram_parameter("v", [S, D_IN], F32, isOutput=False)
    wq_ext = nc.declare_dram_parameter("wq", [D_IN, DK], F32, isOutput=False)
    wk_ext = nc.declare_dram_parameter("wk", [D_IN, DK], F32, isOutput=False)
    wv_ext = nc.declare_dram_parameter("wv", [D_IN, DV], F32, isOutput=False)
    bq_ext = nc.declare_dram_parameter("bq", [DK], F32, isOutput=False)
    bk_ext = nc.declare_dram_parameter("bk", [DK], F32, isOutput=False)
    bv_ext = nc.declare_dram_parameter("bv", [DV], F32, isOutput=False)
    out_ext = nc.declare_dram_parameter("out", [SQ, DV], F32, isOutput=True)

    with tile.TileContext(nc) as tc:
        with (
            tc.tile_pool(name="single", bufs=1) as sg,
            tc.tile_pool(name="xn", bufs=8) as xn,
            tc.tile_pool(name="xtp", bufs=6) as xtp,
            tc.tile_pool(name="exg", bufs=4) as exg,
            tc.tile_pool(name="vtp", bufs=2) as vtp,
            tc.tile_pool(name="fin", bufs=2) as fin,
            tc.tile_pool(name="otp", bufs=2, space="PSUM") as otp,
            tc.tile_pool(name="scp", bufs=2, space="PSUM") as scp,
            tc.tile_pool(name="trp", bufs=3, space="PSUM") as trp,
            tc.tile_pool(name="pjp", bufs=1, space="PSUM") as pjp,
        ):
            # ---- issue the first query loads before anything else so the PE
            # has data as early as possible
            xq_first = []
            for qb in range(2):
                xqt = xn.tile([128, 4, D_IN], BF16, tag="xn", name="xnt")
                for h in range(2):
                    nc.gpsimd.dma_start(
                        out=xqt[:, 2 * h : 2 * h + 2, :],
                        in_=q_ext[
                            512 * qb + 256 * h : 512 * qb + 256 * (h + 1), :
                        ].rearrange("(t p) d -> p t d", p=128),
                    )
                xq_first.append(xqt)

            # ---- constants
            ident_b = sg.tile([128, 128], BF16)
            make_identity(nc, ident_b[:, :])
            ident_f = sg.tile([128, 128], F32)
            make_identity(nc, ident_f[:, :])

            # weights -> bf16 [128, 8, 64] (cast during DMA)
            Wq = sg.tile([128, NCH, DK], BF16)
            Wk = sg.tile([128, NCH, DK], BF16)
            Wv = sg.tile([128, NCH, DV], BF16)
            for W, ext in ((Wq, wq_ext), (Wk, wk_ext), (Wv, wv_ext)):
                nc.gpsimd.dma_start(
                    out=W[:, :, :], in_=ext.rearrange("(c p) n -> p c n", p=128)
                )
            bq_t = sg.tile([64, 1], F32)
            bk_t = sg.tile([64, 1], F32)
            bv_t = sg.tile([64, 1], F32)
            for bt, ext in ((bq_t, bq_ext), (bk_t, bk_ext), (bv_t, bv_ext)):
                nc.sync.dma_start(out=bt[:, :], in_=ext[:].unsqueeze(-1))

            # projected tensors (bf16)
            qT = sg.tile([64, SQ], BF16)    # [dk, q]
            kT = sg.tile([64, S], BF16)     # [dk, kv]
            v1 = sg.tile([128, NKV, DV + 1], BF16)  # v natural + ones col
            nc.vector.memset(v1[:, :, DV : DV + 1], 1.0)
            # exp tiles for query blocks 2-3, PV-ed after the kv stream
            ex2 = sg.tile([128, 2 * NKV, 512], BF16)

            # prime the PE clock
            prime_ps = trp.tile([128, 128], BF16, tag="tr")
            nc.tensor.transpose(prime_ps[:, :], ident_b[:, :], ident_b[:, :])

            # round-robin copy engines for PSUM->SBUF evictions. GPSIMD cannot
            # read PSUM, so split between DVE and Act (Copy shares the Exp
            # activation table set -> no table reloads).
            cp_state = {"i": 0}

            def eng_copy(dst, src):
                i = cp_state["i"]
                cp_state["i"] += 1
                if i % 3 == 2:
                    nc.scalar.copy(dst, src)
                else:
                    nc.vector.tensor_copy(dst, src)

            def load_block(x_ext, s0):
                """One 512-row natural-layout cast load -> [128, 4, 1024].
                Split in two DMAs so the first tiles land sooner."""
                xt = xn.tile([128, 4, D_IN], BF16, tag="xn", name="xnt")
                for h in range(2):
                    nc.gpsimd.dma_start(
                        out=xt[:, 2 * h : 2 * h + 2, :],
                        in_=x_ext[s0 + 256 * h : s0 + 256 * (h + 1), :].rearrange(
                            "(t p) d -> p t d", p=128
                        ),
                    )
                return xt

            def interleave(prod_units, cons_units):
                """Emit producer thunks, sprinkling consumer thunks evenly."""
                np_, nc_ = len(prod_units), len(cons_units)
                ci = 0
                for pi, u in enumerate(prod_units):
                    u()
                    while ci * np_ < (pi + 1) * nc_:
                        cons_units[ci]()
                        ci += 1
                while ci < nc_:
                    cons_units[ci]()
                    ci += 1

            DELAY = 3  # chunks between transpose-group and its projection

            def prod_block(xnt, W, bias_t, outT, col0):
                """Thunks producing outT[:, col0:col0+512] = (x_block W + b)^T
                via PE transposes + chunk-accumulated projection."""
                st = {"pj": None}
                xts = [None] * NCH

                def trans_unit(c):
                    def f():
                        tr = trp.tile([128, 512], BF16, tag="tr", name="tr")
                        xt = xtp.tile([128, 512], BF16, tag="xt", name="xt")
                        for t in range(4):
                            nc.tensor.transpose(
                                tr[:, 128 * t : 128 * (t + 1)],
                                xnt[:, t, 128 * c : 128 * (c + 1)],
                                ident_b[:, :],
                            )
                            if t == 1:
                                eng_copy(xt[:, 0:256], tr[:, 0:256])
                        eng_copy(xt[:, 256:512], tr[:, 256:512])
                        xts[c] = xt

                    return f

                def proj_unit(m):
                    def f():
                        if st["pj"] is None:
                            st["pj"] = pjp.tile([64, 512], F32, tag="pj", name="pj")
                        nc.tensor.matmul(
                            st["pj"][:, :],
                            W[:, m, :],
                            xts[m][:, :],
                            start=(m == 0),
                            stop=(m == NCH - 1),
                        )

                    return f

                def bias_unit():
                    nc.vector.tensor_scalar_add(
                        outT[:, col0 : col0 + 512], st["pj"][:, :], bias_t[:, :]
                    )

                units = []
                for c in range(NCH):
                    units.append(trans_unit(c))
                    if c >= DELAY:
                        units.append(proj_unit(c - DELAY))
                for m in range(NCH - DELAY, NCH):
                    units.append(proj_unit(m))
                units.append(bias_unit)
                return units

            def flips_unit(vt_blk, b):
                """vT block [64, 512] -> natural v1[:, 4b:4b+4, :64]."""

                def f():
                    tr = trp.tile([128, 256], BF16, tag="tr", name="trf")
                    for j in range(4):
                        nc.tensor.transpose(
                            tr[:, 64 * j : 64 * (j + 1)],
                            vt_blk[:, 128 * j : 128 * (j + 1)],
                            ident_b[0:64, 0:64],
                        )
                    nc.vector.tensor_copy(
                        v1[:, 4 * b : 4 * b + 4, 0:DV],
                        tr[:, 0:256].rearrange("p (j v) -> p j v", j=4),
                    )

                return f

            def cons_block(b, ots):
                """Attention thunks for kv block b: scoresT+exp for all 4 q
                blocks, immediate PV for q blocks 0-1."""
                exd = {}
                units = []
                for j in range(4):
                    c = 4 * b + j

                    def sc_unit(c, qb):
                        def f():
                            sp = scp.tile([128, 512], F32, tag="sc", name="sp")
                            nc.tensor.matmul(
                                sp[:, :],
                                kT[:, 128 * c : 128 * (c + 1)],
                                qT[:, 512 * qb : 512 * (qb + 1)],
                                start=True,
                                stop=True,
                            )
                            if qb < 2:
                                ex = exg.tile(
                                    [128, 512], BF16, tag="ex", name="ex"
                                )[:, :]
                            else:
                                ex = ex2[:, 2 * c + (qb - 2), :]
                            nc.scalar.activation(
                                out=ex, in_=sp[:, :], func=EXP, scale=0.125
                            )
                            exd[(c, qb)] = ex

                        return f

                    def pv_unit(c, qb):
                        def f():
                            nc.tensor.matmul(
                                ots[qb][:, :],
                                v1[:, c, :],
                                exd[(c, qb)],
                                start=(c == 0),
                                stop=(c == NKV - 1),
                            )

                        return f

                    for qb in range(NQB):
                        units.append(sc_unit(c, qb))
                    units.append(pv_unit(c, 0))
                    units.append(pv_unit(c, 1))
                return units

            def fin_copy(ot):
                o_sb = fin.tile([DV + 1, 512], F32, tag="osb", name="osb")
                nc.vector.tensor_copy(o_sb[:, :], ot[:, :])
                return o_sb

            def fin_rest_units(o_sb, qb):
                def unit(t):
                    def f():
                        tp = scp.tile([128, DV + 1], F32, tag="sc", name="tp")
                        nc.tensor.transpose(
                            tp[:, :],
                            o_sb[:, 128 * t : 128 * (t + 1)],
                            ident_f[0 : DV + 1, 0 : DV + 1],
                        )
                        rec = fin.tile([128, 1], F32, tag="rec", name="rec")
                        nc.vector.reciprocal(rec[:, :], tp[:, DV : DV + 1])
                        o_f = fin.tile([128, DV], F32, tag="of", name="of")
                        nc.vector.tensor_scalar_mul(o_f[:, :], tp[:, 0:DV], rec[:, :])
                        nc.sync.dma_start(
                            out=out_ext[
                                512 * qb + 128 * t : 512 * qb + 128 * (t + 1), :
                            ],
                            in_=o_f[:, :],
                        )

                    return f

                return [unit(t) for t in range(4)]

            # ---- Q phase: project all 2048 query rows
            for qb in range(NQB):
                xnt = xq_first[qb] if qb < 2 else load_block(q_ext, 512 * qb)
                interleave(prod_block(xnt, Wq, bq_t, qT, 512 * qb), [])

            # ---- KV stream: produce k/v block b while consuming attention of
            # block b-1 (keeps the PE stream dense so it holds peak p-state)
            ots = [
                otp.tile([DV + 1, 512], F32, tag="ot", name=f"ot{i}") for i in range(2)
            ]
            cons = []
            for b in range(NKB):
                xk = load_block(k_ext, 512 * b)
                xv = load_block(v_ext, 512 * b)
                vt = vtp.tile([64, 512], BF16, tag="vt", name="vt")
                prod = (
                    prod_block(xk, Wk, bk_t, kT, 512 * b)
                    + prod_block(xv, Wv, bv_t, vt, 0)
                    + [flips_unit(vt, b)]
                )
                interleave(prod, cons)
                cons = cons_block(b, ots)

            # ---- tail: attention for the last kv block, interleaved with the
            # deferred PV sweep for q blocks 2-3 (chunks not from the last
            # block have their exp tiles ready; transpose banks are free)
            ots2 = [
                trp.tile([DV + 1, 512], F32, tag="tr", name=f"ot2{i}")
                for i in range(2)
            ]

            def g2_pv_unit(c, g):
                def f():
                    nc.tensor.matmul(
                        ots2[g][:, :],
                        v1[:, c, :],
                        ex2[:, 2 * c + g, :],
                        start=(c == 0),
                        stop=(c == NKV - 1),
                    )

                return f

            early = [g2_pv_unit(c, g) for c in range(NKV - 4) for g in range(2)]
            late = [g2_pv_unit(c, g) for c in range(NKV - 4, NKV) for g in range(2)]
            interleave(cons, early)
            o_sb0 = fin_copy(ots[0])
            o_sb1 = fin_copy(ots[1])
            for u in late:
                u()
            for u in fin_rest_units(o_sb0, 0) + fin_rest_units(o_sb1, 1):
                u()
            o_sb2 = fin_copy(ots2[0])
            o_sb3 = fin_copy(ots2[1])
            for u in fin_rest_units(o_sb2, 2) + fin_rest_units(o_sb3, 3):
                u()

    nc.compile()
    return nc


def _get_nc():
    if "nc" not in _NC_CACHE:
        _NC_CACHE["nc"] = build_attention_nc()
    return _NC_CACHE["nc"]


def kernel(query, key, value, Wq, bq, Wk, bk, Wv, bv):
    query = np.asarray(query, dtype=np.float32)
    key = np.asarray(key, dtype=np.float32)
    value = np.asarray(value, dtype=np.float32)
    wq = np.ascontiguousarray(np.asarray(Wq, np.float32))
    wk = np.ascontiguousarray(np.asarray(Wk, np.float32))
    wv = np.ascontiguousarray(np.asarray(Wv, np.float32))
    bq_ = np.ascontiguousarray(np.asarray(bq, np.float32))
    bk_ = np.ascontiguousarray(np.asarray(bk, np.float32))
    bv_ = np.ascontiguousarray(np.asarray(bv, np.float32))

    in_maps = []
    for b in range(B):
        for h in range(2):
            in_maps.append(
                {
                    "q": np.ascontiguousarray(query[b, h * SQ : (h + 1) * SQ]),
                    "k": np.ascontiguousarray(key[b]),
                    "v": np.ascontiguousarray(value[b]),
                    "wq": wq, "wk": wk, "wv": wv,
                    "bq": bq_, "bk": bk_, "bv": bv_,
                }
            )

    nc = _get_nc()
    trace = bool(int(os.environ.get("BASS_KERNEL_TRACE", "0")))
    res = run_bass_kernel_spmd(nc, in_maps, core_ids=list(range(8)), trace=trace)
    _NC_CACHE["last_results"] = res

    out = np.empty((B, S, DV), np.float32)
    for b in range(B):
        for h in range(2):
            out[b, h * SQ : (h + 1) * SQ] = res.results[2 * b + h]["out"]
    return out


# revision 15
# speedup vs baseline: 1.6870x; 1.0064x over previous
"""Trainium2 Bass kernel for nn_AttentionHead (B=4, S=4096, D_IN=1024, DK=DV=64).

Sharding: 8 cores = batch(4) x query-half(2). Each core computes attention for
its 2048 query rows against the full 4096-key sequence of its batch. No
collectives.

Per-core algorithm (matmul compute in bf16, f32 accumulation):
  1. Natural-layout cast-DMA loads (f32 DRAM -> bf16 SBUF, 4KB-contiguous
     rows, descriptor-efficient): x tiles [128 seq, 1024 d].
  2. x^T via PE transposes ([128,128] blocks -> PSUM, engine copy-back to
     SBUF), software-pipelined with the projection matmuls.
  3. Projections W-stationary: qT [64, 2048], kT [64, 4096], vT per-block
     [64, 512]; bias added on PSUM eviction. vT is PE-flipped to natural
     v1 [kv, 65] with a ones column (col 64) so PV also accumulates the
     softmax denominator.
  4. Streaming attention: per kv chunk, scoresT = kT_c^T qT for all 4 query
     blocks; exp via ScalarE (scale=1/8). PV accumulates in PSUM for query
     blocks 0-1 immediately; exp tiles for blocks 2-3 are kept in SBUF and
     their PV runs as a dense sweep afterwards (PSUM has only 2 free banks
     for output accumulators).
  5. Finalize: PE-transpose out^T -> [128 q, 65], reciprocal of col 64,
     per-partition scale, DMA out f32.
"""
import os
import numpy as np

import concourse.bass as bass
import concourse.mybir as mybir
import concourse.tile as tile
from concourse import bacc
from concourse.bass_utils import run_bass_kernel_spmd
from concourse.masks import make_identity

F32 = mybir.dt.float32
BF16 = mybir.dt.bfloat16
EXP = mybir.ActivationFunctionType.Exp

B, S, D_IN, DK, DV = 4, 4096, 1024, 64, 64
SQ = S // 2            # 2048 query rows per core
NCH = D_IN // 128      # 8 d_in chunks
NKV = S // 128         # 32 kv tiles
NQB = SQ // 512        # 4 query blocks of 512
NKB = S // 512         # 8 kv blocks of 512

_NC_CACHE = {}


def build_attention_nc():
    nc = bacc.Bacc()

    q_ext = nc.declare_dram_parameter("q", [SQ, D_IN], F32, isOutput=False)
    k_ext = nc.declare_dram_parameter("k", [S, D_IN], F32, isOutput=False)
    v_ext = nc.declare_dram_parameter("v", [S, D_IN], F32, isOutput=False)
    wq_ext = nc.declare_dram_parameter("wq", [D_IN, DK], F32, isOutput=False)
    wk_ext = nc.declare_dram_parameter("wk", [D_IN, DK], F32, isOutput=False)
    wv_ext = nc.declare_dram_parameter("wv", [D_IN, DV], F32, isOutput=False)
    bq_ext = nc.declare_dram_parameter("bq", [DK], F32, isOutput=False)
    bk_ext = nc.declare_dram_parameter("bk", [DK], F32, isOutput=False)
    bv_ext = nc.declare_dram_parameter("bv", [DV], F32, isOutput=False)
    out_ext = nc.declare_dram_parameter("out", [SQ, DV], F32, isOutput=True)

    with tile.TileContext(nc) as tc:
        with (
            tc.tile_pool(name="single", bufs=1) as sg,
            tc.tile_pool(name="xn", bufs=8) as xn,
            tc.tile_pool(name="xtp", bufs=6) as xtp,
            tc.tile_pool(name="exg", bufs=4) as exg,
            tc.tile_pool(name="vtp", bufs=2) as vtp,
            tc.tile_pool(name="fin", bufs=2) as fin,
            tc.tile_pool(name="otp", bufs=2, space="PSUM") as otp,
            tc.tile_pool(name="scp", bufs=2, space="PSUM") as scp,
            tc.tile_pool(name="trp", bufs=3, space="PSUM") as trp,
            tc.tile_pool(name="pjp", bufs=1, space="PSUM") as pjp,
        ):
            # ---- issue the first query loads before anything else so the PE
            # has data as early as possible
            xq_first = []
            for qb in range(2):
                xqt = xn.tile([128, 4, D_IN], BF16, tag="xn", name="xnt")
                for h in range(2):
                    nc.gpsimd.dma_start(
                        out=xqt[:, 2 * h : 2 * h + 2, :],
                        in_=q_ext[
                            512 * qb + 256 * h : 512 * qb + 256 * (h + 1), :
                        ].rearrange("(t p) d -> p t d", p=128),
                    )
                xq_first.append(xqt)

            # ---- constants
            ident_b = sg.tile([128, 128], BF16)
            make_identity(nc, ident_b[:, :])
            ident_f = sg.tile([128, 128], F32)
            make_identity(nc, ident_f[:, :])

            # weights -> bf16 [128, 8, 64] (cast during DMA)
            Wq = sg.tile([128, NCH, DK], BF16)
            Wk = sg.tile([128, NCH, DK], BF16)
            Wv = sg.tile([128, NCH, DV], BF16)
            for W, ext in ((Wq, wq_ext), (Wk, wk_ext), (Wv, wv_ext)):
                nc.gpsimd.dma_start(
                    out=W[:, :, :], in_=ext.rearrange("(c p) n -> p c n", p=128)
                )
            bq_t = sg.tile([64, 1], F32)
            bk_t = sg.tile([64, 1], F32)
            bv_t = sg.tile([64, 1], F32)
            for bt, ext in ((bq_t, bq_ext), (bk_t, bk_ext), (bv_t, bv_ext)):
                nc.sync.dma_start(out=bt[:, :], in_=ext[:].unsqueeze(-1))

            # projected tensors (bf16)
            qT = sg.tile([64, SQ], BF16)    # [dk, q]
            kT = sg.tile([64, S], BF16)     # [dk, kv]
            v1 = sg.tile([128, NKV, DV + 1], BF16)  # v natural + ones col
            nc.vector.memset(v1[:, :, DV : DV + 1], 1.0)
            # exp tiles for query blocks 2-3, PV-ed after the kv stream
            ex2 = sg.tile([128, 2 * NKV, 512], BF16)

            # prime the PE clock
            prime_ps = trp.tile([128, 128], BF16, tag="tr")
            nc.tensor.transpose(prime_ps[:, :], ident_b[:, :], ident_b[:, :])

            # round-robin copy engines for PSUM->SBUF evictions. GPSIMD cannot
            # read PSUM, so split between DVE and Act (Copy shares the Exp
            # activation table set -> no table reloads).
            cp_state = {"i": 0}

            def eng_copy(dst, src):
                i = cp_state["i"]
                cp_state["i"] += 1
                if i % 3 == 2:
                    nc.scalar.copy(dst, src)
                else:
                    nc.vector.tensor_copy(dst, src)

            def load_block(x_ext, s0):
                """One 512-row natural-layout cast load -> [128, 4, 1024].
                Split in two DMAs so the first tiles land sooner."""
                xt = xn.tile([128, 4, D_IN], BF16, tag="xn", name="xnt")
                for h in range(2):
                    nc.gpsimd.dma_start(
                        out=xt[:, 2 * h : 2 * h + 2, :],
                        in_=x_ext[s0 + 256 * h : s0 + 256 * (h + 1), :].rearrange(
                            "(t p) d -> p t d", p=128
                        ),
                    )
                return xt

            def interleave(prod_units, cons_units):
                """Emit producer thunks, sprinkling consumer thunks evenly."""
                np_, nc_ = len(prod_units), len(cons_units)
                ci = 0
                for pi, u in enumerate(prod_units):
                    u()
                    while ci * np_ < (pi + 1) * nc_:
                        cons_units[ci]()
                        ci += 1
                while ci < nc_:
                    cons_units[ci]()
                    ci += 1

            DELAY = 3  # chunks between transpose-group and its projection

            def prod_block(xnt, W, bias_t, outT, col0):
                """Thunks producing outT[:, col0:col0+512] = (x_block W + b)^T
                via PE transposes + chunk-accumulated projection."""
                st = {"pj": None}
                xts = [None] * NCH

                def trans_unit(c):
                    def f():
                        tr = trp.tile([128, 512], BF16, tag="tr", name="tr")
                        xt = xtp.tile([128, 512], BF16, tag="xt", name="xt")
                        for t in range(4):
                            nc.tensor.transpose(
                                tr[:, 128 * t : 128 * (t + 1)],
                                xnt[:, t, 128 * c : 128 * (c + 1)],
                                ident_b[:, :],
                            )
                            if t == 1:
                                eng_copy(xt[:, 0:256], tr[:, 0:256])
                        eng_copy(xt[:, 256:512], tr[:, 256:512])
                        xts[c] = xt

                    return f

                def proj_unit(m):
                    def f():
                        if st["pj"] is None:
                            st["pj"] = pjp.tile([64, 512], F32, tag="pj", name="pj")
                        nc.tensor.matmul(
                            st["pj"][:, :],
                            W[:, m, :],
                            xts[m][:, :],
                            start=(m == 0),
                            stop=(m == NCH - 1),
                        )

                    return f

                def bias_unit():
                    nc.vector.tensor_scalar_add(
                        outT[:, col0 : col0 + 512], st["pj"][:, :], bias_t[:, :]
                    )

                units = []
                for c in range(NCH):
                    units.append(trans_unit(c))
                    if c >= DELAY:
                        units.append(proj_unit(c - DELAY))
                for m in range(NCH - DELAY, NCH):
                    units.append(proj_unit(m))
                units.append(bias_unit)
                return units

            def flips_unit(vt_blk, b):
                """vT block [64, 512] -> natural v1[:, 4b:4b+4, :64]."""

                def f():
                    tr = trp.tile([128, 256], BF16, tag="tr", name="trf")
                    for j in range(4):
                        nc.tensor.transpose(
                            tr[:, 64 * j : 64 * (j + 1)],
                            vt_blk[:, 128 * j : 128 * (j + 1)],
                            ident_b[0:64, 0:64],
                        )
                    nc.vector.tensor_copy(
                        v1[:, 4 * b : 4 * b + 4, 0:DV],
                        tr[:, 0:256].rearrange("p (j v) -> p j v", j=4),
                    )

                return f

            def cons_block(b, ots):
                """Attention thunks for kv block b: scoresT+exp for all 4 q
                blocks, immediate PV for q blocks 0-1."""
                exd = {}
                units = []
                for j in range(4):
                    c = 4 * b + j

                    def sc_unit(c, qb):
                        def f():
                            sp = scp.tile([128, 512], F32, tag="sc", name="sp")
                            nc.tensor.matmul(
                                sp[:, :],
                                kT[:, 128 * c : 128 * (c + 1)],
                                qT[:, 512 * qb : 512 * (qb + 1)],
                                start=True,
                                stop=True,
                            )
                            if qb < 2:
                                ex = exg.tile(
                                    [128, 512], BF16, tag="ex", name="ex"
                                )[:, :]
                            else:
                                ex = ex2[:, 2 * c + (qb - 2), :]
                            nc.scalar.activation(
                                out=ex, in_=sp[:, :], func=EXP, scale=0.125
                            )
                            exd[(c, qb)] = ex

                        return f

                    def pv_unit(c, qb):
                        def f():
                            nc.tensor.matmul(
                                ots[qb][:, :],
                                v1[:, c, :],
                                exd[(c, qb)],
                                start=(c == 0),
                                stop=(c == NKV - 1),
                            )

                        return f

                    for qb in range(NQB):
                        units.append(sc_unit(c, qb))
                    units.append(pv_unit(c, 0))
                    units.append(pv_unit(c, 1))
                return units

            def fin_copy(ot):
                o_sb = fin.tile([DV + 1, 512], F32, tag="osb", name="osb")
                nc.vector.tensor_copy(o_sb[:, :], ot[:, :])
                return o_sb

            def fin_rest_units(o_sb, qb, pools=None):
                def unit(t):
                    def f():
                        pool = (pools or [scp])[t % len(pools or [scp])]
                        tp = pool.tile(
                            [128, DV + 1],
                            F32,
                            tag="sc" if pool is scp else "tr",
                            name="tp",
                        )
                        nc.tensor.transpose(
                            tp[:, :],
                            o_sb[:, 128 * t : 128 * (t + 1)],
                            ident_f[0 : DV + 1, 0 : DV + 1],
                        )
                        rec = fin.tile([128, 1], F32, tag="rec", name="rec")
                        nc.vector.reciprocal(rec[:, :], tp[:, DV : DV + 1])
                        o_f = fin.tile([128, DV], F32, tag="of", name="of")
                        nc.vector.tensor_scalar_mul(o_f[:, :], tp[:, 0:DV], rec[:, :])
                        nc.sync.dma_start(
                            out=out_ext[
                                512 * qb + 128 * t : 512 * qb + 128 * (t + 1), :
                            ],
                            in_=o_f[:, :],
                        )

                    return f

                return [unit(t) for t in range(4)]

            # ---- Q phase: project all 2048 query rows
            for qb in range(NQB):
                xnt = xq_first[qb] if qb < 2 else load_block(q_ext, 512 * qb)
                interleave(prod_block(xnt, Wq, bq_t, qT, 512 * qb), [])

            # ---- KV stream: produce k/v block b while consuming attention of
            # block b-1 (keeps the PE stream dense so it holds peak p-state)
            ots = [
                otp.tile([DV + 1, 512], F32, tag="ot", name=f"ot{i}") for i in range(2)
            ]
            cons = []
            for b in range(NKB):
                xk = load_block(k_ext, 512 * b)
                xv = load_block(v_ext, 512 * b)
                vt = vtp.tile([64, 512], BF16, tag="vt", name="vt")
                prod = (
                    prod_block(xk, Wk, bk_t, kT, 512 * b)
                    + prod_block(xv, Wv, bv_t, vt, 0)
                    + [flips_unit(vt, b)]
                )
                interleave(prod, cons)
                cons = cons_block(b, ots)

            # ---- tail: attention for the last kv block, interleaved with the
            # deferred PV sweep for q blocks 2-3 (chunks not from the last
            # block have their exp tiles ready; transpose banks are free)
            ots2 = [
                trp.tile([DV + 1, 512], F32, tag="tr", name=f"ot2{i}")
                for i in range(2)
            ]

            def g2_pv_unit(c, g):
                def f():
                    nc.tensor.matmul(
                        ots2[g][:, :],
                        v1[:, c, :],
                        ex2[:, 2 * c + g, :],
                        start=(c == 0),
                        stop=(c == NKV - 1),
                    )

                return f

            early = [g2_pv_unit(c, g) for c in range(NKV - 4) for g in range(2)]
            late = [g2_pv_unit(c, g) for c in range(NKV - 4, NKV) for g in range(2)]
            interleave(cons, early)
            o_sb0 = fin_copy(ots[0])
            o_sb1 = fin_copy(ots[1])
            # finalize math for q blocks 0-1 rides inside the remaining PV sweep
            r01 = [u for pair in zip(
                fin_rest_units(o_sb0, 0), fin_rest_units(o_sb1, 1)
            ) for u in pair]
            interleave(late, r01)
            o_sb2 = fin_copy(ots2[0])
            o_sb3 = fin_copy(ots2[1])
            r23 = [u for pair in zip(
                fin_rest_units(o_sb2, 2, [scp, trp]),
                fin_rest_units(o_sb3, 3, [scp, trp]),
            ) for u in pair]
            for u in r23:
                u()

    nc.compile()
    return nc


def _get_nc():
    if "nc" not in _NC_CACHE:
        _NC_CACHE["nc"] = build_attention_nc()
    return _NC_CACHE["nc"]


def kernel(query, key, value, Wq, bq, Wk, bk, Wv, bv):
    query = np.asarray(query, dtype=np.float32)
    key = np.asarray(key, dtype=np.float32)
    value = np.asarray(value, dtype=np.float32)
    wq = np.ascontiguousarray(np.asarray(Wq, np.float32))
    wk = np.ascontiguousarray(np.asarray(Wk, np.float32))
    wv = np.ascontiguousarray(np.asarray(Wv, np.float32))
    bq_ = np.ascontiguousarray(np.asarray(bq, np.float32))
    bk_ = np.ascontiguousarray(np.asarray(bk, np.float32))
    bv_ = np.ascontiguousarray(np.asarray(bv, np.float32))

    in_maps = []
    for b in range(B):
        for h in range(2):
            in_maps.append(
                {
                    "q": np.ascontiguousarray(query[b, h * SQ : (h + 1) * SQ]),
                    "k": np.ascontiguousarray(key[b]),
                    "v": np.ascontiguousarray(value[b]),
                    "wq": wq, "wk": wk, "wv": wv,
                    "bq": bq_, "bk": bk_, "bv": bv_,
                }
            )

    nc = _get_nc()
    trace = bool(int(os.environ.get("BASS_KERNEL_TRACE", "0")))
    res = run_bass_kernel_spmd(nc, in_maps, core_ids=list(range(8)), trace=trace)
    _NC_CACHE["last_results"] = res

    out = np.empty((B, S, DV), np.float32)
    for b in range(B):
        for h in range(2):
            out[b, h * SQ : (h + 1) * SQ] = res.results[2 * b + h]["out"]
    return out


# revision 18
# speedup vs baseline: 1.6987x; 1.0069x over previous
"""Trainium2 Bass kernel for nn_AttentionHead (B=4, S=4096, D_IN=1024, DK=DV=64).

Sharding: 8 cores = batch(4) x query-half(2). Each core computes attention for
its 2048 query rows against the full 4096-key sequence of its batch. No
collectives.

Per-core algorithm (matmul compute in bf16, f32 accumulation):
  1. Natural-layout cast-DMA loads (f32 DRAM -> bf16 SBUF, 4KB-contiguous
     rows, descriptor-efficient): x tiles [128 seq, 1024 d].
  2. x^T via PE transposes ([128,128] blocks -> PSUM, engine copy-back to
     SBUF), software-pipelined with the projection matmuls.
  3. Projections W-stationary: qT [64, 2048], kT [64, 4096], vT per-block
     [64, 512]; bias added on PSUM eviction. vT is PE-flipped to natural
     v1 [kv, 65] with a ones column (col 64) so PV also accumulates the
     softmax denominator.
  4. Streaming attention: per kv chunk, scoresT = kT_c^T qT for all 4 query
     blocks; exp via ScalarE (scale=1/8). PV accumulates in PSUM for query
     blocks 0-1 immediately; exp tiles for blocks 2-3 are kept in SBUF and
     their PV runs as a dense sweep afterwards (PSUM has only 2 free banks
     for output accumulators).
  5. Finalize: PE-transpose out^T -> [128 q, 65], reciprocal of col 64,
     per-partition scale, DMA out f32.
"""
import os
import numpy as np

import concourse.bass as bass
import concourse.mybir as mybir
import concourse.tile as tile
from concourse import bacc
from concourse.bass_utils import run_bass_kernel_spmd
from concourse.masks import make_identity

F32 = mybir.dt.float32
BF16 = mybir.dt.bfloat16
EXP = mybir.ActivationFunctionType.Exp

B, S, D_IN, DK, DV = 4, 4096, 1024, 64, 64
SQ = S // 2            # 2048 query rows per core
NCH = D_IN // 128      # 8 d_in chunks
NKV = S // 128         # 32 kv tiles
NQB = SQ // 512        # 4 query blocks of 512
NKB = S // 512         # 8 kv blocks of 512

_NC_CACHE = {}


def build_attention_nc():
    nc = bacc.Bacc()

    q_ext = nc.declare_dram_parameter("q", [SQ, D_IN], F32, isOutput=False)
    k_ext = nc.declare_dram_parameter("k", [S, D_IN], F32, isOutput=False)
    v_ext = nc.declare_dram_parameter("v", [S, D_IN], F32, isOutput=False)
    wq_ext = nc.declare_dram_parameter("wq", [D_IN, DK], F32, isOutput=False)
    wk_ext = nc.declare_dram_parameter("wk", [D_IN, DK], F32, isOutput=False)
    wv_ext = nc.declare_dram_parameter("wv", [D_IN, DV], F32, isOutput=False)
    bq_ext = nc.declare_dram_parameter("bq", [DK], F32, isOutput=False)
    bk_ext = nc.declare_dram_parameter("bk", [DK], F32, isOutput=False)
    bv_ext = nc.declare_dram_parameter("bv", [DV], F32, isOutput=False)
    out_ext = nc.declare_dram_parameter("out", [SQ, DV], F32, isOutput=True)

    with tile.TileContext(nc) as tc:
        with (
            tc.tile_pool(name="single", bufs=1) as sg,
            tc.tile_pool(name="xn", bufs=9) as xn,
            tc.tile_pool(name="xtp", bufs=6) as xtp,
            tc.tile_pool(name="exg", bufs=4) as exg,
            tc.tile_pool(name="vtp", bufs=2) as vtp,
            tc.tile_pool(name="fin", bufs=2) as fin,
            tc.tile_pool(name="otp", bufs=2, space="PSUM") as otp,
            tc.tile_pool(name="scp", bufs=2, space="PSUM") as scp,
            tc.tile_pool(name="trp", bufs=3, space="PSUM") as trp,
            tc.tile_pool(name="pjp", bufs=1, space="PSUM") as pjp,
        ):
            # ---- issue the first query loads before anything else so the PE
            # has data as early as possible
            xq_first = []
            for qb in range(2):
                xqt = xn.tile([128, 4, D_IN], BF16, tag="xn", name="xnt")
                for h in range(2):
                    nc.gpsimd.dma_start(
                        out=xqt[:, 2 * h : 2 * h + 2, :],
                        in_=q_ext[
                            512 * qb + 256 * h : 512 * qb + 256 * (h + 1), :
                        ].rearrange("(t p) d -> p t d", p=128),
                    )
                xq_first.append(xqt)

            # ---- constants
            ident_b = sg.tile([128, 128], BF16)
            make_identity(nc, ident_b[:, :])
            ident_f = sg.tile([128, 128], F32)
            make_identity(nc, ident_f[:, :])

            # weights -> bf16 [128, 8, 64] (cast during DMA)
            Wq = sg.tile([128, NCH, DK], BF16)
            Wk = sg.tile([128, NCH, DK], BF16)
            Wv = sg.tile([128, NCH, DV], BF16)
            for W, ext in ((Wq, wq_ext), (Wk, wk_ext), (Wv, wv_ext)):
                nc.gpsimd.dma_start(
                    out=W[:, :, :], in_=ext.rearrange("(c p) n -> p c n", p=128)
                )
            bq_t = sg.tile([64, 1], F32)
            bk_t = sg.tile([64, 1], F32)
            bv_t = sg.tile([64, 1], F32)
            for bt, ext in ((bq_t, bq_ext), (bk_t, bk_ext), (bv_t, bv_ext)):
                nc.sync.dma_start(out=bt[:, :], in_=ext[:].unsqueeze(-1))

            # projected tensors (bf16)
            qT = sg.tile([64, SQ], BF16)    # [dk, q]
            kT = sg.tile([64, S], BF16)     # [dk, kv]
            v1 = sg.tile([128, NKV, DV + 1], BF16)  # v natural + ones col
            nc.vector.memset(v1[:, :, DV : DV + 1], 1.0)
            # exp tiles for query blocks 2-3, PV-ed after the kv stream
            ex2 = sg.tile([128, 2 * NKV, 512], BF16)

            # prime the PE clock
            prime_ps = trp.tile([128, 128], BF16, tag="tr")
            nc.tensor.transpose(prime_ps[:, :], ident_b[:, :], ident_b[:, :])

            # round-robin copy engines for PSUM->SBUF evictions. GPSIMD cannot
            # read PSUM, so split between DVE and Act (Copy shares the Exp
            # activation table set -> no table reloads).
            cp_state = {"i": 0}

            def eng_copy(dst, src):
                i = cp_state["i"]
                cp_state["i"] += 1
                if i % 3 == 2:
                    nc.scalar.copy(dst, src)
                else:
                    nc.vector.tensor_copy(dst, src)

            def load_block(x_ext, s0):
                """One 512-row natural-layout cast load -> [128, 4, 1024].
                Split in two DMAs so the first tiles land sooner."""
                xt = xn.tile([128, 4, D_IN], BF16, tag="xn", name="xnt")
                for h in range(2):
                    nc.gpsimd.dma_start(
                        out=xt[:, 2 * h : 2 * h + 2, :],
                        in_=x_ext[s0 + 256 * h : s0 + 256 * (h + 1), :].rearrange(
                            "(t p) d -> p t d", p=128
                        ),
                    )
                return xt

            def interleave(prod_units, cons_units, lead=0):
                """Emit producer thunks, sprinkling consumer thunks evenly.
                `lead` consumers are emitted up-front (bridges the DMA wait at
                a block boundary)."""
                np_, nc_ = len(prod_units), len(cons_units)
                ci = 0
                while ci < min(lead, nc_):
                    cons_units[ci]()
                    ci += 1
                for pi, u in enumerate(prod_units):
                    u()
                    while ci < nc_ and ci - lead < (pi + 1) * (nc_ - lead) // np_:
                        cons_units[ci]()
                        ci += 1
                while ci < nc_:
                    cons_units[ci]()
                    ci += 1

            DELAY = 3  # chunks between transpose-group and its projection

            def prod_block(xnt, W, bias_t, outT, col0):
                """Thunks producing outT[:, col0:col0+512] = (x_block W + b)^T
                via PE transposes + chunk-accumulated projection."""
                st = {"pj": None}
                xts = [None] * NCH

                def trans_unit(c):
                    def f():
                        tr = trp.tile([128, 512], BF16, tag="tr", name="tr")
                        xt = xtp.tile([128, 512], BF16, tag="xt", name="xt")
                        for t in range(4):
                            nc.tensor.transpose(
                                tr[:, 128 * t : 128 * (t + 1)],
                                xnt[:, t, 128 * c : 128 * (c + 1)],
                                ident_b[:, :],
                            )
                            if t == 1:
                                eng_copy(xt[:, 0:256], tr[:, 0:256])
                        eng_copy(xt[:, 256:512], tr[:, 256:512])
                        xts[c] = xt

                    return f

                def proj_unit(m):
                    def f():
                        if st["pj"] is None:
                            st["pj"] = pjp.tile([64, 512], F32, tag="pj", name="pj")
                        nc.tensor.matmul(
                            st["pj"][:, :],
                            W[:, m, :],
                            xts[m][:, :],
                            start=(m == 0),
                            stop=(m == NCH - 1),
                        )

                    return f

                def bias_unit():
                    nc.vector.tensor_scalar_add(
                        outT[:, col0 : col0 + 512], st["pj"][:, :], bias_t[:, :]
                    )

                units = []
                for c in range(NCH):
                    units.append(trans_unit(c))
                    if c >= DELAY:
                        units.append(proj_unit(c - DELAY))
                for m in range(NCH - DELAY, NCH):
                    units.append(proj_unit(m))
                units.append(bias_unit)
                return units

            def flips_unit(vt_blk, b):
                """vT block [64, 512] -> natural v1[:, 4b:4b+4, :64]."""

                def f():
                    tr = trp.tile([128, 256], BF16, tag="tr", name="trf")
                    for j in range(4):
                        nc.tensor.transpose(
                            tr[:, 64 * j : 64 * (j + 1)],
                            vt_blk[:, 128 * j : 128 * (j + 1)],
                            ident_b[0:64, 0:64],
                        )
                    nc.vector.tensor_copy(
                        v1[:, 4 * b : 4 * b + 4, 0:DV],
                        tr[:, 0:256].rearrange("p (j v) -> p j v", j=4),
                    )

                return f

            def cons_block(b, ots):
                """Attention thunks for kv block b: scoresT+exp for all 4 q
                blocks, immediate PV for q blocks 0-1."""
                exd = {}
                units = []
                for j in range(4):
                    c = 4 * b + j

                    def sc_unit(c, qb):
                        def f():
                            sp = scp.tile([128, 512], F32, tag="sc", name="sp")
                            nc.tensor.matmul(
                                sp[:, :],
                                kT[:, 128 * c : 128 * (c + 1)],
                                qT[:, 512 * qb : 512 * (qb + 1)],
                                start=True,
                                stop=True,
                            )
                            if qb < 2:
                                ex = exg.tile(
                                    [128, 512], BF16, tag="ex", name="ex"
                                )[:, :]
                            else:
                                ex = ex2[:, 2 * c + (qb - 2), :]
                            nc.scalar.activation(
                                out=ex, in_=sp[:, :], func=EXP, scale=0.125
                            )
                            exd[(c, qb)] = ex

                        return f

                    def pv_unit(c, qb):
                        def f():
                            nc.tensor.matmul(
                                ots[qb][:, :],
                                v1[:, c, :],
                                exd[(c, qb)],
                                start=(c == 0),
                                stop=(c == NKV - 1),
                            )

                        return f

                    for qb in range(NQB):
                        units.append(sc_unit(c, qb))
                    units.append(pv_unit(c, 0))
                    units.append(pv_unit(c, 1))
                return units

            def fin_copy(ot):
                o_sb = fin.tile([DV + 1, 512], F32, tag="osb", name="osb")
                nc.vector.tensor_copy(o_sb[:, :], ot[:, :])
                return o_sb

            def fin_rest_units(o_sb, qb, pools=None):
                def unit(t):
                    def f():
                        pool = (pools or [scp])[t % len(pools or [scp])]
                        tp = pool.tile(
                            [128, DV + 1],
                            F32,
                            tag="sc" if pool is scp else "tr",
                            name="tp",
                        )
                        nc.tensor.transpose(
                            tp[:, :],
                            o_sb[:, 128 * t : 128 * (t + 1)],
                            ident_f[0 : DV + 1, 0 : DV + 1],
                        )
                        rec = fin.tile([128, 1], F32, tag="rec", name="rec")
                        nc.vector.reciprocal(rec[:, :], tp[:, DV : DV + 1])
                        o_f = fin.tile([128, DV], F32, tag="of", name="of")
                        nc.vector.tensor_scalar_mul(o_f[:, :], tp[:, 0:DV], rec[:, :])
                        nc.sync.dma_start(
                            out=out_ext[
                                512 * qb + 128 * t : 512 * qb + 128 * (t + 1), :
                            ],
                            in_=o_f[:, :],
                        )

                    return f

                return [unit(t) for t in range(4)]

            # ---- Q phase: project all 2048 query rows
            for qb in range(NQB):
                xnt = xq_first[qb] if qb < 2 else load_block(q_ext, 512 * qb)
                interleave(prod_block(xnt, Wq, bq_t, qT, 512 * qb), [])

            # ---- KV stream: produce k/v block b while consuming attention of
            # block b-1 (keeps the PE stream dense so it holds peak p-state)
            ots = [
                otp.tile([DV + 1, 512], F32, tag="ot", name=f"ot{i}") for i in range(2)
            ]
            cons = []
            for b in range(NKB):
                xk = load_block(k_ext, 512 * b)
                xv = load_block(v_ext, 512 * b)
                vt = vtp.tile([64, 512], BF16, tag="vt", name="vt")
                prod = (
                    prod_block(xk, Wk, bk_t, kT, 512 * b)
                    + prod_block(xv, Wv, bv_t, vt, 0)
                    + [flips_unit(vt, b)]
                )
                interleave(prod, cons, lead=4)
                cons = cons_block(b, ots)

            # ---- tail: attention for the last kv block, interleaved with the
            # deferred PV sweep for q blocks 2-3 (chunks not from the last
            # block have their exp tiles ready; transpose banks are free)
            ots2 = [
                trp.tile([DV + 1, 512], F32, tag="tr", name=f"ot2{i}")
                for i in range(2)
            ]

            def g2_pv_unit(c, g):
                def f():
                    nc.tensor.matmul(
                        ots2[g][:, :],
                        v1[:, c, :],
                        ex2[:, 2 * c + g, :],
                        start=(c == 0),
                        stop=(c == NKV - 1),
                    )

                return f

            early = [g2_pv_unit(c, g) for c in range(NKV - 4) for g in range(2)]
            late = [g2_pv_unit(c, g) for c in range(NKV - 4, NKV) for g in range(2)]
            interleave(cons, early)
            o_sb0 = fin_copy(ots[0])
            o_sb1 = fin_copy(ots[1])
            # finalize math for q blocks 0-1 rides inside the remaining PV sweep
            r01 = [u for pair in zip(
                fin_rest_units(o_sb0, 0), fin_rest_units(o_sb1, 1)
            ) for u in pair]
            interleave(late, r01)
            o_sb2 = fin_copy(ots2[0])
            o_sb3 = fin_copy(ots2[1])
            r23 = [u for pair in zip(
                fin_rest_units(o_sb2, 2, [scp, trp]),
                fin_rest_units(o_sb3, 3, [scp, trp]),
            ) for u in pair]
            for u in r23:
                u()

    nc.compile()
    return nc


def _get_nc():
    if "nc" not in _NC_CACHE:
        _NC_CACHE["nc"] = build_attention_nc()
    return _NC_CACHE["nc"]


def kernel(query, key, value, Wq, bq, Wk, bk, Wv, bv):
    query = np.asarray(query, dtype=np.float32)
    key = np.asarray(key, dtype=np.float32)
    value = np.asarray(value, dtype=np.float32)
    wq = np.ascontiguousarray(np.asarray(Wq, np.float32))
    wk = np.ascontiguousarray(np.asarray(Wk, np.float32))
    wv = np.ascontiguousarray(np.asarray(Wv, np.float32))
    bq_ = np.ascontiguousarray(np.asarray(bq, np.float32))
    bk_ = np.ascontiguousarray(np.asarray(bk, np.float32))
    bv_ = np.ascontiguousarray(np.asarray(bv, np.float32))

    in_maps = []
    for b in range(B):
        for h in range(2):
            in_maps.append(
                {
                    "q": np.ascontiguousarray(query[b, h * SQ : (h + 1) * SQ]),
                    "k": np.ascontiguousarray(key[b]),
                    "v": np.ascontiguousarray(value[b]),
                    "wq": wq, "wk": wk, "wv": wv,
                    "bq": bq_, "bk": bk_, "bv": bv_,
                }
            )

    nc = _get_nc()
    trace = bool(int(os.environ.get("BASS_KERNEL_TRACE", "0")))
    res = run_bass_kernel_spmd(nc, in_maps, core_ids=list(range(8)), trace=trace)
    _NC_CACHE["last_results"] = res

    out = np.empty((B, S, DV), np.float32)
    for b in range(B):
        for h in range(2):
            out[b, h * SQ : (h + 1) * SQ] = res.results[2 * b + h]["out"]
    return out


# revision 19
# speedup vs baseline: 1.8107x; 1.0659x over previous
"""Trainium2 Bass kernel for nn_AttentionHead (B=4, S=4096, D_IN=1024, DK=DV=64).

Sharding: 8 cores = batch(4) x query-half(2). Each core computes attention for
its 2048 query rows against the full 4096-key sequence of its batch. No
collectives.

Per-core algorithm (matmul compute in bf16, f32 accumulation):
  1. Natural-layout cast-DMA loads (f32 DRAM -> bf16 SBUF, 4KB-contiguous
     rows, descriptor-efficient): x tiles [128 seq, 1024 d].
  2. x^T via PE transposes ([128,128] blocks -> PSUM, engine copy-back to
     SBUF), software-pipelined with the projection matmuls.
  3. Projections W-stationary: qT [64, 2048], kT [64, 4096], vT per-block
     [64, 512]; bias added on PSUM eviction. vT is PE-flipped to natural
     v1 [kv, 65] with a ones column (col 64) so PV also accumulates the
     softmax denominator.
  4. Streaming attention: per kv chunk, scoresT = kT_c^T qT for all 4 query
     blocks; exp via ScalarE (scale=1/8). PV accumulates in PSUM for query
     blocks 0-1 immediately; exp tiles for blocks 2-3 are kept in SBUF and
     their PV runs as a dense sweep afterwards (PSUM has only 2 free banks
     for output accumulators).
  5. Finalize: PE-transpose out^T -> [128 q, 65], reciprocal of col 64,
     per-partition scale, DMA out f32.
"""
import os
import numpy as np

import concourse.bass as bass
import concourse.mybir as mybir
import concourse.tile as tile
from concourse import bacc
from concourse.bass_utils import run_bass_kernel_spmd
from concourse.masks import make_identity

F32 = mybir.dt.float32
BF16 = mybir.dt.bfloat16
EXP = mybir.ActivationFunctionType.Exp

B, S, D_IN, DK, DV = 4, 4096, 1024, 64, 64
SQ = S // 2            # 2048 query rows per core
NCH = D_IN // 128      # 8 d_in chunks
NKV = S // 128         # 32 kv tiles
NQB = SQ // 512        # 4 query blocks of 512
NKB = S // 512         # 8 kv blocks of 512

_NC_CACHE = {}


def build_attention_nc():
    nc = bacc.Bacc()

    q_ext = nc.declare_dram_parameter("q", [SQ, D_IN], F32, isOutput=False)
    k_ext = nc.declare_dram_parameter("k", [S, D_IN], F32, isOutput=False)
    v_ext = nc.declare_dram_parameter("v", [S, D_IN], F32, isOutput=False)
    wq_ext = nc.declare_dram_parameter("wq", [D_IN, DK], F32, isOutput=False)
    wk_ext = nc.declare_dram_parameter("wk", [D_IN, DK], F32, isOutput=False)
    wv_ext = nc.declare_dram_parameter("wv", [D_IN, DV], F32, isOutput=False)
    bq_ext = nc.declare_dram_parameter("bq", [DK], F32, isOutput=False)
    bk_ext = nc.declare_dram_parameter("bk", [DK], F32, isOutput=False)
    bv_ext = nc.declare_dram_parameter("bv", [DV], F32, isOutput=False)
    out_ext = nc.declare_dram_parameter("out", [SQ, DV], F32, isOutput=True)

    with tile.TileContext(nc) as tc:
        with (
            tc.tile_pool(name="single", bufs=1) as sg,
            tc.tile_pool(name="xn", bufs=9) as xn,
            tc.tile_pool(name="xtp", bufs=6) as xtp,
            tc.tile_pool(name="exg", bufs=4) as exg,
            tc.tile_pool(name="vtp", bufs=2) as vtp,
            tc.tile_pool(name="fin", bufs=2) as fin,
            tc.tile_pool(name="otp", bufs=2, space="PSUM") as otp,
            tc.tile_pool(name="scp", bufs=2, space="PSUM") as scp,
            tc.tile_pool(name="trp", bufs=3, space="PSUM") as trp,
            tc.tile_pool(name="pjp", bufs=1, space="PSUM") as pjp,
        ):
            # ---- issue the first query loads before anything else so the PE
            # has data as early as possible
            xq_first = []
            for qb in range(2):
                xqt = xn.tile([128, 4, D_IN], BF16, tag="xn", name="xnt")
                for h in range(2):
                    nc.gpsimd.dma_start(
                        out=xqt[:, 2 * h : 2 * h + 2, :],
                        in_=q_ext[
                            512 * qb + 256 * h : 512 * qb + 256 * (h + 1), :
                        ].rearrange("(t p) d -> p t d", p=128),
                    )
                xq_first.append(xqt)

            # ---- constants
            ident_b = sg.tile([128, 128], BF16)
            make_identity(nc, ident_b[:, :])
            ident_f = sg.tile([128, 128], F32)
            make_identity(nc, ident_f[:, :])

            # weights -> bf16 [128, 8, 64] (cast during DMA)
            Wq = sg.tile([128, NCH, DK], BF16)
            Wk = sg.tile([128, NCH, DK], BF16)
            Wv = sg.tile([128, NCH, DV], BF16)
            for W, ext in ((Wq, wq_ext), (Wk, wk_ext), (Wv, wv_ext)):
                nc.gpsimd.dma_start(
                    out=W[:, :, :], in_=ext.rearrange("(c p) n -> p c n", p=128)
                )
            bq_t = sg.tile([64, 1], F32)
            bk_t = sg.tile([64, 1], F32)
            bv_t = sg.tile([64, 1], F32)
            for bt, ext in ((bq_t, bq_ext), (bk_t, bk_ext), (bv_t, bv_ext)):
                nc.sync.dma_start(out=bt[:, :], in_=ext[:].unsqueeze(-1))

            # projected tensors (bf16)
            qT = sg.tile([64, SQ], BF16)    # [dk, q]
            kT = sg.tile([64, S], BF16)     # [dk, kv]
            v1 = sg.tile([128, NKV, DV + 1], BF16)  # v natural + ones col
            nc.vector.memset(v1[:, :, DV : DV + 1], 1.0)
            # exp tiles for query blocks 2-3, PV-ed after the kv stream
            ex2 = sg.tile([128, 2 * NKV, 512], BF16)

            # prime the PE clock
            prime_ps = trp.tile([128, 128], BF16, tag="tr")
            nc.tensor.transpose(prime_ps[:, :], ident_b[:, :], ident_b[:, :])

            # round-robin copy engines for PSUM->SBUF evictions. GPSIMD cannot
            # read PSUM, so split between DVE and Act (Copy shares the Exp
            # activation table set -> no table reloads).
            cp_state = {"i": 0}

            def eng_copy(dst, src):
                i = cp_state["i"]
                cp_state["i"] += 1
                if i % 3 == 2:
                    nc.scalar.copy(dst, src)
                else:
                    nc.vector.tensor_copy(dst, src)

            def load_block(x_ext, s0):
                """One 512-row natural-layout cast load -> [128, 4, 1024].
                Split in two DMAs so the first tiles land sooner."""
                xt = xn.tile([128, 4, D_IN], BF16, tag="xn", name="xnt")
                for h in range(2):
                    nc.gpsimd.dma_start(
                        out=xt[:, 2 * h : 2 * h + 2, :],
                        in_=x_ext[s0 + 256 * h : s0 + 256 * (h + 1), :].rearrange(
                            "(t p) d -> p t d", p=128
                        ),
                    )
                return xt

            def interleave(prod_units, cons_units, lead=0):
                """Emit producer thunks, sprinkling consumer thunks evenly.
                `lead` consumers are emitted up-front (bridges the DMA wait at
                a block boundary)."""
                np_, nc_ = len(prod_units), len(cons_units)
                ci = 0
                while ci < min(lead, nc_):
                    cons_units[ci]()
                    ci += 1
                for pi, u in enumerate(prod_units):
                    u()
                    while ci < nc_ and ci - lead < (pi + 1) * (nc_ - lead) // np_:
                        cons_units[ci]()
                        ci += 1
                while ci < nc_:
                    cons_units[ci]()
                    ci += 1

            DELAY = 3  # chunks between transpose-group and its projection

            def prod_block(xnt, W, bias_t, outT, col0):
                """Thunks producing outT[:, col0:col0+512] = (x_block W + b)^T
                via PE transposes + chunk-accumulated projection."""
                st = {"pj": None}
                xts = [None] * NCH

                def trans_unit(c):
                    def f():
                        tr = trp.tile([128, 512], BF16, tag="tr", name="tr")
                        xt = xtp.tile([128, 512], BF16, tag="xt", name="xt")
                        for t in range(4):
                            nc.tensor.transpose(
                                tr[:, 128 * t : 128 * (t + 1)],
                                xnt[:, t, 128 * c : 128 * (c + 1)],
                                ident_b[:, :],
                            )
                        eng_copy(xt[:, :], tr[:, :])
                        xts[c] = xt

                    return f

                def proj_unit(m):
                    def f():
                        if st["pj"] is None:
                            st["pj"] = pjp.tile([64, 512], F32, tag="pj", name="pj")
                        nc.tensor.matmul(
                            st["pj"][:, :],
                            W[:, m, :],
                            xts[m][:, :],
                            start=(m == 0),
                            stop=(m == NCH - 1),
                        )

                    return f

                def bias_unit():
                    nc.vector.tensor_scalar_add(
                        outT[:, col0 : col0 + 512], st["pj"][:, :], bias_t[:, :]
                    )

                units = []
                for c in range(NCH):
                    units.append(trans_unit(c))
                    if c >= DELAY:
                        units.append(proj_unit(c - DELAY))
                for m in range(NCH - DELAY, NCH):
                    units.append(proj_unit(m))
                units.append(bias_unit)
                return units

            def flips_unit(vt_blk, b):
                """vT block [64, 512] -> natural v1[:, 4b:4b+4, :64]."""

                def f():
                    tr = trp.tile([128, 256], BF16, tag="tr", name="trf")
                    for j in range(4):
                        nc.tensor.transpose(
                            tr[:, 64 * j : 64 * (j + 1)],
                            vt_blk[:, 128 * j : 128 * (j + 1)],
                            ident_b[0:64, 0:64],
                        )
                    nc.vector.tensor_copy(
                        v1[:, 4 * b : 4 * b + 4, 0:DV],
                        tr[:, 0:256].rearrange("p (j v) -> p j v", j=4),
                    )

                return f

            def cons_block(b, ots):
                """Attention thunks for kv block b: scoresT+exp for all 4 q
                blocks, immediate PV for q blocks 0-1."""
                exd = {}
                units = []
                for j in range(4):
                    c = 4 * b + j

                    def sc_unit(c, qb):
                        def f():
                            sp = scp.tile([128, 512], F32, tag="sc", name="sp")
                            nc.tensor.matmul(
                                sp[:, :],
                                kT[:, 128 * c : 128 * (c + 1)],
                                qT[:, 512 * qb : 512 * (qb + 1)],
                                start=True,
                                stop=True,
                            )
                            if qb < 2:
                                ex = exg.tile(
                                    [128, 512], BF16, tag="ex", name="ex"
                                )[:, :]
                            else:
                                ex = ex2[:, 2 * c + (qb - 2), :]
                            nc.scalar.activation(
                                out=ex, in_=sp[:, :], func=EXP, scale=0.125
                            )
                            exd[(c, qb)] = ex

                        return f

                    def pv_unit(c, qb):
                        def f():
                            nc.tensor.matmul(
                                ots[qb][:, :],
                                v1[:, c, :],
                                exd[(c, qb)],
                                start=(c == 0),
                                stop=(c == NKV - 1),
                            )

                        return f

                    for qb in range(NQB):
                        units.append(sc_unit(c, qb))
                    units.append(pv_unit(c, 0))
                    units.append(pv_unit(c, 1))
                return units

            def fin_copy(ot):
                o_sb = fin.tile([DV + 1, 512], F32, tag="osb", name="osb")
                nc.vector.tensor_copy(o_sb[:, :], ot[:, :])
                return o_sb

            def fin_rest_units(o_sb, qb, pools=None):
                def unit(t):
                    def f():
                        pool = (pools or [scp])[t % len(pools or [scp])]
                        tp = pool.tile(
                            [128, DV + 1],
                            F32,
                            tag="sc" if pool is scp else "tr",
                            name="tp",
                        )
                        nc.tensor.transpose(
                            tp[:, :],
                            o_sb[:, 128 * t : 128 * (t + 1)],
                            ident_f[0 : DV + 1, 0 : DV + 1],
                        )
                        rec = fin.tile([128, 1], F32, tag="rec", name="rec")
                        nc.vector.reciprocal(rec[:, :], tp[:, DV : DV + 1])
                        o_f = fin.tile([128, DV], F32, tag="of", name="of")
                        nc.vector.tensor_scalar_mul(o_f[:, :], tp[:, 0:DV], rec[:, :])
                        nc.sync.dma_start(
                            out=out_ext[
                                512 * qb + 128 * t : 512 * qb + 128 * (t + 1), :
                            ],
                            in_=o_f[:, :],
                        )

                    return f

                return [unit(t) for t in range(4)]

            # ---- Q phase: project all 2048 query rows
            for qb in range(NQB):
                xnt = xq_first[qb] if qb < 2 else load_block(q_ext, 512 * qb)
                interleave(prod_block(xnt, Wq, bq_t, qT, 512 * qb), [])

            # ---- KV stream: produce k/v block b while consuming attention of
            # block b-1 (keeps the PE stream dense so it holds peak p-state)
            ots = [
                otp.tile([DV + 1, 512], F32, tag="ot", name=f"ot{i}") for i in range(2)
            ]
            cons = []
            for b in range(NKB):
                xk = load_block(k_ext, 512 * b)
                xv = load_block(v_ext, 512 * b)
                vt = vtp.tile([64, 512], BF16, tag="vt", name="vt")
                prod = (
                    prod_block(xk, Wk, bk_t, kT, 512 * b)
                    + prod_block(xv, Wv, bv_t, vt, 0)
                    + [flips_unit(vt, b)]
                )
                interleave(prod, cons, lead=4)
                cons = cons_block(b, ots)

            # ---- tail: attention for the last kv block, interleaved with the
            # deferred PV sweep for q blocks 2-3 (chunks not from the last
            # block have their exp tiles ready; transpose banks are free)
            ots2 = [
                trp.tile([DV + 1, 512], F32, tag="tr", name=f"ot2{i}")
                for i in range(2)
            ]

            def g2_pv_unit(c, g):
                def f():
                    nc.tensor.matmul(
                        ots2[g][:, :],
                        v1[:, c, :],
                        ex2[:, 2 * c + g, :],
                        start=(c == 0),
                        stop=(c == NKV - 1),
                    )

                return f

            early = [g2_pv_unit(c, g) for c in range(NKV - 4) for g in range(2)]
            late = [g2_pv_unit(c, g) for c in range(NKV - 4, NKV) for g in range(2)]
            interleave(cons, early)
            o_sb0 = fin_copy(ots[0])
            o_sb1 = fin_copy(ots[1])
            # finalize math for q blocks 0-1 rides inside the remaining PV sweep
            r01 = [u for pair in zip(
                fin_rest_units(o_sb0, 0), fin_rest_units(o_sb1, 1)
            ) for u in pair]
            interleave(late, r01)
            o_sb2 = fin_copy(ots2[0])
            o_sb3 = fin_copy(ots2[1])
            r23 = [u for pair in zip(
                fin_rest_units(o_sb2, 2, [scp, trp]),
                fin_rest_units(o_sb3, 3, [scp, trp]),
            ) for u in pair]
            for u in r23:
                u()

    nc.compile()
    return nc


def _get_nc():
    if "nc" not in _NC_CACHE:
        _NC_CACHE["nc"] = build_attention_nc()
    return _NC_CACHE["nc"]


def kernel(query, key, value, Wq, bq, Wk, bk, Wv, bv):
    query = np.asarray(query, dtype=np.float32)
    key = np.asarray(key, dtype=np.float32)
    value = np.asarray(value, dtype=np.float32)
    wq = np.ascontiguousarray(np.asarray(Wq, np.float32))
    wk = np.ascontiguousarray(np.asarray(Wk, np.float32))
    wv = np.ascontiguousarray(np.asarray(Wv, np.float32))
    bq_ = np.ascontiguousarray(np.asarray(bq, np.float32))
    bk_ = np.ascontiguousarray(np.asarray(bk, np.float32))
    bv_ = np.ascontiguousarray(np.asarray(bv, np.float32))

    in_maps = []
    for b in range(B):
        for h in range(2):
            in_maps.append(
                {
                    "q": np.ascontiguousarray(query[b, h * SQ : (h + 1) * SQ]),
                    "k": np.ascontiguousarray(key[b]),
                    "v": np.ascontiguousarray(value[b]),
                    "wq": wq, "wk": wk, "wv": wv,
                    "bq": bq_, "bk": bk_, "bv": bv_,
                }
            )

    nc = _get_nc()
    trace = bool(int(os.environ.get("BASS_KERNEL_TRACE", "0")))
    res = run_bass_kernel_spmd(nc, in_maps, core_ids=list(range(8)), trace=trace)
    _NC_CACHE["last_results"] = res

    out = np.empty((B, S, DV), np.float32)
    for b in range(B):
        for h in range(2):
            out[b, h * SQ : (h + 1) * SQ] = res.results[2 * b + h]["out"]
    return out
